# revision 17
# baseline (speedup 1.0000x reference)
"""Trainium2 Bass kernel for a 2-layer edge-featured GAT + mean-pool + FC.

Sharding: 256 graphs split 32-per-core across 8 cores (batch is sorted, so
each core owns a contiguous, graph-aligned node range). Edges live on the
core that owns their destination; per-core node tables are AllGathered
between layers so any core can gather arbitrary source rows.

v2 design (vs the fp32 SWDGE baseline):
- Node tables are bf16. Layer-1 rows are 384 bf16 (768B):
  [h head0(64) | 1 | h1(64) | 1 | h2(64) | 1 | h3(64) | 1 | asrc(4) |
   adst(4) | pad]; the interleaved 1-columns produce softmax denominators
  through the same scatter matmul. Layer-2 rows are 256 bf16 (512B).
- Src-row gathers are SWDGE dma_gather on 4 rotating queues (the Q7
  descriptor generator stalls on ring space with one queue; four queues
  nearly double throughput). Dst-side gathers are gone entirely:
- The edge->dst one-hot matrices (and their transposes) are HOST-PRECOMPUTED
  bf16 constants (the edge structure is input data, not device data).
  oh[128e, 128j] drives the segment-sum scatter matmul; ohT[128j, 128e]
  broadcasts per-dst-node logits to edges via a tiny K=128 matmul
  (adp = ohT^T @ adst_tile).
- p = exp(leaky_relu(asrc+adst+aedge)) is computed per edge (max of two
  exps), multiplied into the message rows including the 1-columns, so one
  matmul per 128-edge chunk accumulates both Sum(p*h) and Sum(p).
  Normalization happens once per node in the epilogue (exactly equivalent
  to the reference's softmax; the max-shift cancels in the ratio).
- Mean-pool one-hot and 1/count are host constants; final FC as in v1.
"""

import sys

sys.path.insert(0, "/opt/trn_rl_repo")

import math
from contextlib import ExitStack

import numpy as np
import ml_dtypes

import concourse.bacc as bacc
import concourse.bass as bass
import concourse.mybir as mybir
import concourse.tile as tile
from concourse.bass_utils import run_bass_kernel_spmd
from concourse.masks import make_identity

P = 128
NCORES = 8
BF = ml_dtypes.bfloat16

FULL_CFG = dict(N=20000, E=640000, FIN=128, HID=64, HEADS=4, NG=256, OUT=32)

F32 = mybir.dt.float32
BF16 = mybir.dt.bfloat16
I16 = mybir.dt.int16

# layer-1 row layout (bf16): 4 x [h(64) | 1] then asrc(4) adst(4) pad -> 384
CW = 65                  # head group width (64 + denom column)
D1R = 4 * CW             # 260
ASRC1, ADST1 = D1R, D1R + 4
ROW1 = 384
# layer-2 row layout (bf16): [h(64) | 1 | asrc(1) | adst(1) | pad] -> 256
ASRC2, ADST2 = CW, CW + 1
ROW2 = 256


# ---------------------------------------------------------------------------
# Host-side preparation: integer index manipulation + array reordering only.
# ---------------------------------------------------------------------------
def prepare(inputs, cfg):
    N, E, FIN, HID, HEADS, NG, OUT = (
        cfg["N"], cfg["E"], cfg["FIN"], cfg["HID"], cfg["HEADS"], cfg["NG"],
        cfg["OUT"],
    )
    GPC = NG // NCORES  # graphs per core

    x = np.asarray(inputs["x"], np.float32)
    ei = np.asarray(inputs["edge_index"], np.int64)
    ea = np.asarray(inputs["edge_attr"], np.float32)
    batch = np.asarray(inputs["batch"], np.int64)
    src, dst = ei[0], ei[1]

    # node ranges per core (graph-aligned; batch is sorted)
    bounds = np.searchsorted(batch, np.arange(NCORES + 1) * GPC)
    node_cnt = np.diff(bounds)
    NT = max(1, math.ceil(node_cnt.max() / P))
    NSLICE = NT * P
    NROWS = NCORES * NSLICE
    assert NROWS < 32768, f"int16 gather index overflow: {NROWS}"

    core_of_node = np.minimum(batch // GPC, NCORES - 1).astype(np.int64)
    rowid = np.empty(N, np.int64)
    for c in range(NCORES):
        ns, ne = bounds[c], bounds[c + 1]
        rowid[ns:ne] = c * NSLICE + np.arange(ne - ns)

    # edges sorted by dst; core blocks are contiguous
    order = np.argsort(dst, kind="stable")
    dsts = dst[order]
    srcs = src[order]
    ws = ea[order, 0]
    ecore = core_of_node[dsts]
    ebounds = np.searchsorted(ecore, np.arange(NCORES + 1))

    # chunks-per-tile: max over all (core, tile), rounded up to even
    cpt_max = 1
    tile_edge_counts = []
    for c in range(NCORES):
        es, ee = ebounds[c], ebounds[c + 1]
        dln = dsts[es:ee] - bounds[c]
        tid = dln // P
        cnts = np.bincount(tid, minlength=NT)
        tile_edge_counts.append(cnts)
        if len(cnts):
            cpt_max = max(cpt_max, math.ceil(cnts.max() / P))
    CPT = cpt_max + (cpt_max % 2)  # even
    CPT = max(CPT, 2)
    CH = CPT // 2
    NCHUNK = NT * CPT

    # per-layer-1/2 attention-edge scalars (tiny float prep, host-replicated)
    q1 = (np.asarray(inputs["We1"], np.float32).reshape(HEADS, HID)
          * np.asarray(inputs["att_edge1"], np.float32)).sum(axis=1)  # [H]
    q2 = float((np.asarray(inputs["We2"], np.float32).reshape(-1)
                * np.asarray(inputs["att_edge2"], np.float32).reshape(-1))
               .sum())

    jj = np.arange(P, dtype=np.int64)

    per_core = []
    for c in range(NCORES):
        ns, ne = bounds[c], bounds[c + 1]
        es, ee = ebounds[c], ebounds[c + 1]
        nloc = ne - ns

        xs = np.zeros((NSLICE, FIN), np.float32)
        xs[:nloc] = x[ns:ne]

        srcrow = np.zeros((NT, CPT * P), np.int64)
        dstl = np.full((NT, CPT * P), -1, np.int64)
        wv = np.zeros((NT, CPT * P), np.float32)

        dln = dsts[es:ee] - ns
        tid = dln // P
        cnts = tile_edge_counts[c]
        off = np.zeros(NT + 1, np.int64)
        off[1:NT + 1] = np.cumsum(cnts[:NT])
        for t in range(NT):
            k = int(cnts[t]) if t < len(cnts) else 0
            if k == 0:
                continue
            sel = slice(es + int(off[t]), es + int(off[t]) + k)
            srcrow[t, :k] = rowid[srcs[sel]]
            dstl[t, :k] = dln[int(off[t]):int(off[t]) + k] % P
            wv[t, :k] = ws[sel]

        # one-hot constants: eq[k, e, j] = (dstl[k*128+e] == j); fp8e4 (exact
        # 0/1) halves the constant DMA traffic vs bf16
        dstl_f = dstl.reshape(NCHUNK, P)
        eq = (dstl_f[:, :, None] == jj[None, None, :])
        oh_dev = np.ascontiguousarray(
            eq.transpose(1, 0, 2).reshape(P, NCHUNK * P)).astype(
                ml_dtypes.float8_e4m3)
        ohT_dev = np.ascontiguousarray(
            eq.transpose(2, 0, 1).reshape(P, NCHUNK * P)).astype(
                ml_dtypes.float8_e4m3)

        # per-edge attention-edge terms (w_e * q_h); pad slots -> 0
        wflat = wv.reshape(NCHUNK, P)
        ae1 = np.ascontiguousarray(
            (wflat[:, :, None] * q1[None, None, :]).transpose(1, 0, 2)
        ).astype(BF)                                   # [128, NCHUNK, H]
        ae2 = np.ascontiguousarray(
            (wflat * q2).transpose(1, 0)).astype(BF)   # [128, NCHUNK]

        def wrap_idx(arr):  # [NT, CPT*P] -> [128, NT*CPT*8] int16
            blocks = []
            for t in range(NT):
                for h in range(2):
                    ids = arr[t, h * CH * P:(h + 1) * CH * P]
                    a = ids.reshape(CH * 8, 16).T  # [16, CH*8]
                    blocks.append(np.tile(a, (8, 1)))
            return np.ascontiguousarray(
                np.concatenate(blocks, axis=1)).astype(np.int16)

        # pooling one-hot + 1/count (host: index data only)
        bl = np.full((NSLICE,), -1, np.int64)
        bl[:nloc] = batch[ns:ne] - c * GPC
        poolg = np.ascontiguousarray(
            (bl.reshape(NT, P)[:, :, None] ==
             np.arange(GPC)[None, None, :]).transpose(1, 0, 2)).astype(BF)
        cnt = np.bincount(bl[:nloc], minlength=GPC).astype(np.float32)
        invc = (1.0 / np.maximum(cnt, 1.0)).reshape(GPC, 1).astype(np.float32)

        per_core.append(dict(
            xs=xs, idxs=wrap_idx(srcrow), oh=oh_dev, ohT=ohT_dev,
            ae1=ae1, ae2=ae2, poolg=poolg, invc=invc,
        ))

    # weight-side constants (tiny, host-replicated)
    W1 = np.asarray(inputs["W1"], np.float32)            # [FIN, H*HID]
    W2 = np.asarray(inputs["W2"], np.float32)            # [H*HID, HID]
    rep = lambda vv: np.tile(np.asarray(vv, np.float32).reshape(1, -1),
                             (P, 1)).copy()
    consts = dict(
        W1b=W1.astype(BF),
        W2b=np.ascontiguousarray(
            W2.reshape(2, P, HID).transpose(1, 0, 2)).astype(BF),
        as1b=rep(inputs["att_src1"]), ad1b=rep(inputs["att_dst1"]),
        b1b=rep(inputs["b1"]),
        as2b=rep(inputs["att_src2"]), ad2b=rep(inputs["att_dst2"]),
        b2b=rep(inputs["b2"]),
        fcw=np.asarray(inputs["fcW"], np.float32),
        fcbb=rep(inputs["fcb"]),
    )

    in_maps = []
    for c in range(NCORES):
        m = dict(per_core[c])
        m.update(consts)
        in_maps.append(m)

    meta = dict(NT=NT, CPT=CPT, CH=CH, NSLICE=NSLICE, NROWS=NROWS,
                GPC=GPC, NCHUNK=NCHUNK, **cfg)
    return in_maps, meta


# ---------------------------------------------------------------------------
# Device program.
# ---------------------------------------------------------------------------
def build(meta, reps=1, num_devices=NCORES):
    NT, CPT, CH = meta["NT"], meta["CPT"], meta["CH"]
    NSLICE, NROWS, GPC = meta["NSLICE"], meta["NROWS"], meta["GPC"]
    FIN, HID, HEADS, OUT = meta["FIN"], meta["HID"], meta["HEADS"], meta["OUT"]
    NCHUNK = meta["NCHUNK"]
    D1 = HEADS * HID          # 256
    NI = CH * P               # idxs per gather call
    NIc = NI // 16            # idx columns per call
    A = mybir.AluOpType
    ACT = mybir.ActivationFunctionType
    X = mybir.AxisListType.X
    rg = [list(range(NCORES))]

    nc = bacc.Bacc("TRN2", target_bir_lowering=False, debug=False,
                   num_devices=num_devices,
                   dynamic_dma_scratch_size=65536,
                   num_swdge_queues=4)

    def din(name, shape, dtype=F32):
        return nc.dram_tensor(name, list(shape), dtype,
                              kind="ExternalInput").ap()

    F8 = mybir.dt.float8e4
    xs = din("xs", (NSLICE, FIN))
    idxs_d = din("idxs", (P, NCHUNK * 8), I16)
    oh_d = din("oh", (P, NCHUNK * P), F8)
    ohT_d = din("ohT", (P, NCHUNK * P), F8)
    ae1_d = din("ae1", (P, NCHUNK * HEADS), BF16)
    ae2_d = din("ae2", (P, NCHUNK), BF16)
    poolg_d = din("poolg", (P, NT * GPC), BF16)
    invc_d = din("invc", (GPC, 1))
    W1_d = din("W1b", (FIN, D1), BF16)
    W2_d = din("W2b", (P, 2 * HID), BF16)
    as1_d = din("as1b", (P, D1))
    ad1_d = din("ad1b", (P, D1))
    b1_d = din("b1b", (P, D1))
    as2_d = din("as2b", (P, HID))
    ad2_d = din("ad2b", (P, HID))
    b2_d = din("b2b", (P, HID))
    fcw_d = din("fcw", (HID, OUT))
    fcb_d = din("fcbb", (P, OUT))

    out_d = nc.dram_tensor("out", [GPC, OUT], F32, kind="ExternalOutput").ap()

    gq = [0]  # rotating SWDGE queue

    with tile.TileContext(nc) as tc, ExitStack() as st:
        constp = st.enter_context(tc.tile_pool(name="constp", bufs=1))
        drp = st.enter_context(tc.tile_pool(name="drp", bufs=1, space="DRAM"))

        identf = constp.tile([P, P], F32)
        make_identity(nc, identf[:])
        identb = constp.tile([P, P], BF16)
        make_identity(nc, identb[:])
        ixs_all = constp.tile([P, NCHUNK * 8], I16)
        nc.sync.dma_start(ixs_all[:], idxs_d[:])
        ae1_sb = constp.tile([P, NCHUNK, HEADS], BF16)
        nc.sync.dma_start(ae1_sb[:],
                          ae1_d[:].rearrange("p (k h) -> p k h", h=HEADS))
        ae2_sb = constp.tile([P, NCHUNK], BF16)
        nc.sync.dma_start(ae2_sb[:], ae2_d[:])
        poolg_sb = constp.tile([P, NT, GPC], BF16)
        nc.sync.dma_start(poolg_sb[:],
                          poolg_d[:].rearrange("p (t g) -> p t g", g=GPC))
        invc_sb = constp.tile([GPC, 1], F32)
        nc.sync.dma_start(invc_sb[:], invc_d[:])
        b1_sb = constp.tile([P, D1], F32)
        nc.sync.dma_start(b1_sb[:], b1_d[:])

        for _rep in range(reps):
            t1loc = drp.tile([NSLICE, ROW1], BF16, name=f"t1loc{_rep}")
            t1full = drp.tile([NROWS, ROW1], BF16, addr_space="Shared",
                              name=f"t1full{_rep}")
            t2loc = drp.tile([NSLICE, ROW2], BF16, name=f"t2loc{_rep}")
            t2full = drp.tile([NROWS, ROW2], BF16, addr_space="Shared",
                              name=f"t2full{_rep}")

            adst1_all = constp.tile([P, NT, HEADS], BF16,
                                    name=f"adst1_{_rep}")
            adst2_all = constp.tile([P, NT, 1], BF16, name=f"adst2_{_rep}")
            out1 = constp.tile([P, NT, D1], BF16, name=f"out1_{_rep}")

            # ------------- Phase 0: h1 = x @ W1, logits, table1 -------------
            with tc.tile_pool(name="ph0", bufs=1) as sp, \
                 tc.tile_pool(name="ph0b", bufs=2) as sp2, \
                 tc.tile_pool(name="ph0p", bufs=2, space="PSUM") as pp:
                w1_sb = sp.tile([P, D1], BF16)
                nc.sync.dma_start(w1_sb[:], W1_d[:])
                as1_sb = sp.tile([P, D1], F32)
                nc.sync.dma_start(as1_sb[:], as1_d[:])
                ad1_sb = sp.tile([P, D1], F32)
                nc.sync.dma_start(ad1_sb[:], ad1_d[:])
                xall = sp.tile([P, NT, FIN], F32)
                nc.sync.dma_start(xall[:],
                                  xs[:].rearrange("(t p) f -> p t f", p=P))
                xb = sp.tile([P, NT, FIN], BF16)
                nc.vector.tensor_copy(out=xb[:], in_=xall[:])
                for t in range(NT):
                    xT_ps = pp.tile([P, P], BF16, space="PSUM")
                    nc.tensor.transpose(xT_ps[:], xb[:, t, :], identb[:])
                    xT = sp2.tile([P, P], BF16)
                    nc.vector.tensor_copy(out=xT[:], in_=xT_ps[:])
                    h_ps = pp.tile([P, D1], F32, space="PSUM")
                    nc.tensor.matmul(h_ps[:], lhsT=xT[:], rhs=w1_sb[:],
                                     start=True, stop=True)
                    tmp = sp2.tile([P, D1], F32)
                    red = sp2.tile([P, HEADS], F32)
                    nc.vector.tensor_tensor(out=tmp[:], in0=h_ps[:],
                                            in1=as1_sb[:], op=A.mult)
                    nc.vector.tensor_reduce(
                        out=red[:],
                        in_=tmp[:].rearrange("p (h f) -> p h f", h=HEADS),
                        axis=X, op=A.add)
                    t1t = sp2.tile([P, ROW1], BF16)
                    nc.vector.tensor_copy(out=t1t[:, ASRC1:ASRC1 + HEADS],
                                          in_=red[:])
                    nc.vector.tensor_tensor(out=tmp[:], in0=h_ps[:],
                                            in1=ad1_sb[:], op=A.mult)
                    nc.vector.tensor_reduce(
                        out=red[:],
                        in_=tmp[:].rearrange("p (h f) -> p h f", h=HEADS),
                        axis=X, op=A.add)
                    nc.vector.tensor_copy(out=adst1_all[:, t, :], in_=red[:])
                    nc.vector.tensor_copy(out=t1t[:, ADST1:ADST1 + HEADS],
                                          in_=red[:])
                    hv = t1t[:, 0:D1R].rearrange("p (h f) -> p h f", f=CW)
                    nc.vector.tensor_copy(
                        out=hv[:, :, 0:HID],
                        in_=h_ps[:].rearrange("p (h f) -> p h f", f=HID))
                    nc.vector.memset(hv[:, :, HID:CW], 1.0)
                    nc.vector.memset(t1t[:, ADST1 + HEADS:ROW1], 0.0)
                    nc.sync.dma_start(t1loc[t * P:(t + 1) * P, :], t1t[:])
                nc.gpsimd.collective_compute(
                    "AllGather", A.bypass, replica_groups=rg,
                    ins=[t1loc[:]], outs=[t1full[:]])

            # ------------- Phase 1+2: layer-1 edge phase fused with the
            # per-tile layer-2 GEMM/table build (overlaps AG2 prep) ----------
            with tc.tile_pool(name="p1g", bufs=3) as pg, \
                 tc.tile_pool(name="p1o", bufs=3) as po, \
                 tc.tile_pool(name="p1w", bufs=2) as pw, \
                 tc.tile_pool(name="p1c", bufs=1) as p1c, \
                 tc.tile_pool(name="p1b", bufs=2) as sp2, \
                 tc.tile_pool(name="p1ps", bufs=2, space="PSUM") as pps, \
                 tc.tile_pool(name="p1pa", bufs=2, space="PSUM") as ppa, \
                 tc.tile_pool(name="p1p2", bufs=1, space="PSUM") as pp2:
                w2_sb = p1c.tile([P, 2, HID], BF16)
                nc.sync.dma_start(w2_sb[:],
                                  W2_d[:].rearrange("p (k n) -> p k n", k=2))
                as2_sb = p1c.tile([P, HID], F32)
                nc.sync.dma_start(as2_sb[:], as2_d[:])
                ad2_sb = p1c.tile([P, HID], F32)
                nc.sync.dma_start(ad2_sb[:], ad2_d[:])
                for t in range(NT):
                    acc = pps.tile([P, D1R], F32, space="PSUM")
                    for hh in range(2):
                        kb = t * CPT + hh * CH       # chunk base
                        gbase = (t * 2 + hh) * NIc
                        G = pg.tile([P, CH, ROW1], BF16)
                        nc.gpsimd.dma_gather(
                            G[:], t1full[:], ixs_all[:, gbase:gbase + NIc],
                            NI, NI, ROW1, single_packet=False,
                            queue_num=gq[0] % 4)
                        gq[0] += 1
                        oh_sb = po.tile([P, CH, P], F8)
                        nc.sync.dma_start(
                            oh_sb[:], oh_d[:, kb * P:(kb + CH) * P]
                            .rearrange("p (c j) -> p c j", j=P))
                        ohT_sb = po.tile([P, CH, P], F8)
                        nc.sync.dma_start(
                            ohT_sb[:], ohT_d[:, kb * P:(kb + CH) * P]
                            .rearrange("p (c j) -> p c j", j=P))
                        adp_ps = ppa.tile([P, CH, HEADS], F32, space="PSUM")
                        for c in range(CH):
                            nc.tensor.matmul(adp_ps[:, c, :],
                                             lhsT=ohT_sb[:, c, :],
                                             rhs=adst1_all[:, t, :],
                                             start=True, stop=True)
                        alpha = pw.tile([P, CH, HEADS], F32)
                        nc.vector.tensor_tensor(
                            out=alpha[:], in0=G[:, :, ASRC1:ASRC1 + HEADS],
                            in1=adp_ps[:], op=A.add)
                        nc.vector.tensor_tensor(
                            out=alpha[:], in0=alpha[:],
                            in1=ae1_sb[:, kb:kb + CH, :], op=A.add)
                        e2 = pw.tile([P, CH, HEADS], F32)
                        nc.scalar.activation(out=e2[:], in_=alpha[:],
                                             func=ACT.Exp, scale=0.2)
                        nc.scalar.activation(out=alpha[:], in_=alpha[:],
                                             func=ACT.Exp)
                        p_bf = pw.tile([P, CH, HEADS], BF16)
                        nc.vector.tensor_tensor(out=p_bf[:], in0=alpha[:],
                                                in1=e2[:], op=A.max)
                        gv = G[:, :, 0:D1R].rearrange(
                            "p c (h f) -> p c h f", f=CW)
                        nc.vector.tensor_tensor(
                            out=gv, in0=gv,
                            in1=p_bf[:].unsqueeze(3)
                                .to_broadcast([P, CH, HEADS, CW]),
                            op=A.mult)
                        for c in range(CH):
                            nc.tensor.matmul(
                                acc[:], lhsT=oh_sb[:, c, :],
                                rhs=G[:, c, 0:D1R],
                                start=(hh == 0 and c == 0),
                                stop=(hh == 1 and c == CH - 1))
                    # epilogue: out1 = relu(acc_h / denom_h + b1), fp32 ops
                    # then one contiguous cast (strided bf16 DVE writes
                    # measured pathologically slow on HW)
                    accv = acc[:].rearrange("p (h f) -> p h f", f=CW)
                    dn = pw.tile([P, HEADS], F32)
                    nc.vector.tensor_scalar(out=dn[:], in0=accv[:, :, HID],
                                            scalar1=1e-16, scalar2=None,
                                            op0=A.add)
                    rc = pw.tile([P, HEADS], F32)
                    nc.vector.reciprocal(rc[:], dn[:])
                    o1 = pw.tile([P, HEADS, HID], F32)
                    nc.vector.tensor_tensor(
                        out=o1[:], in0=accv[:, :, 0:HID],
                        in1=rc[:].unsqueeze(2).to_broadcast([P, HEADS, HID]),
                        op=A.mult)
                    nc.vector.tensor_tensor(
                        out=o1[:], in0=o1[:],
                        in1=b1_sb[:].rearrange("p (h f) -> p h f", h=HEADS),
                        op=A.add)
                    nc.vector.tensor_scalar(out=o1[:], in0=o1[:],
                                            scalar1=0.0, scalar2=None,
                                            op0=A.max)
                    nc.vector.tensor_copy(
                        out=out1[:, t, :],
                        in_=o1[:].rearrange("p h f -> p (h f)"))

                    # layer-2 GEMM + table row for this tile (fused phase 2)
                    h2_ps = pp2.tile([P, HID], F32, space="PSUM")
                    for k in range(2):
                        hT_ps = pp2.tile([P, P], BF16, space="PSUM")
                        nc.tensor.transpose(
                            hT_ps[:], out1[:, t, k * P:(k + 1) * P],
                            identb[:])
                        hT = sp2.tile([P, P], BF16)
                        nc.vector.tensor_copy(out=hT[:], in_=hT_ps[:])
                        nc.tensor.matmul(h2_ps[:], lhsT=hT[:],
                                         rhs=w2_sb[:, k, :],
                                         start=(k == 0), stop=(k == 1))
                    t2t = sp2.tile([P, ROW2], BF16)
                    tmp = sp2.tile([P, HID], F32)
                    red1 = sp2.tile([P, 1], F32)
                    nc.vector.tensor_tensor(out=tmp[:], in0=h2_ps[:],
                                            in1=as2_sb[:], op=A.mult)
                    nc.vector.tensor_reduce(out=red1[:],
                                            in_=tmp[:], axis=X, op=A.add)
                    nc.vector.tensor_copy(out=t2t[:, ASRC2:ASRC2 + 1],
                                          in_=red1[:])
                    nc.vector.tensor_tensor(out=tmp[:], in0=h2_ps[:],
                                            in1=ad2_sb[:], op=A.mult)
                    red2 = sp2.tile([P, 1], F32)
                    nc.vector.tensor_reduce(out=red2[:],
                                            in_=tmp[:], axis=X, op=A.add)
                    nc.vector.tensor_copy(out=adst2_all[:, t, :], in_=red2[:])
                    nc.vector.tensor_copy(out=t2t[:, ADST2:ADST2 + 1],
                                          in_=red2[:])
                    nc.vector.tensor_copy(out=t2t[:, 0:HID], in_=h2_ps[:])
                    nc.vector.memset(t2t[:, HID:HID + 1], 1.0)
                    nc.vector.memset(t2t[:, ADST2 + 1:ROW2], 0.0)
                    nc.sync.dma_start(t2loc[t * P:(t + 1) * P, :], t2t[:])
                nc.gpsimd.collective_compute(
                    "AllGather", A.bypass, replica_groups=rg,
                    ins=[t2loc[:]], outs=[t2full[:]])

            # ------------- Phase 3: layer-2 edge phase + pooling ------------
            with tc.tile_pool(name="p3g", bufs=3) as pg, \
                 tc.tile_pool(name="p3o", bufs=3) as po, \
                 tc.tile_pool(name="p3w", bufs=2) as pw, \
                 tc.tile_pool(name="p3c", bufs=1) as pc, \
                 tc.tile_pool(name="p3ps", bufs=2, space="PSUM") as pps, \
                 tc.tile_pool(name="p3pa", bufs=2, space="PSUM") as ppa, \
                 tc.tile_pool(name="p3f", bufs=1, space="PSUM") as ppf, \
                 tc.tile_pool(name="p3pl", bufs=1, space="PSUM") as ppl:
                b2_sb = pc.tile([P, HID], F32)
                nc.sync.dma_start(b2_sb[:], b2_d[:])
                pool_ps = ppl.tile([GPC, HID], F32, space="PSUM")
                for t in range(NT):
                    acc = pps.tile([P, CW], F32, space="PSUM")
                    for hh in range(2):
                        kb = t * CPT + hh * CH
                        gbase = (t * 2 + hh) * NIc
                        G = pg.tile([P, CH, ROW2], BF16)
                        nc.gpsimd.dma_gather(
                            G[:], t2full[:], ixs_all[:, gbase:gbase + NIc],
                            NI, NI, ROW2, single_packet=False,
                            queue_num=gq[0] % 4)
                        gq[0] += 1
                        oh_sb = po.tile([P, CH, P], F8)
                        nc.sync.dma_start(
                            oh_sb[:], oh_d[:, kb * P:(kb + CH) * P]
                            .rearrange("p (c j) -> p c j", j=P))
                        ohT_sb = po.tile([P, CH, P], F8)
                        nc.sync.dma_start(
                            ohT_sb[:], ohT_d[:, kb * P:(kb + CH) * P]
                            .rearrange("p (c j) -> p c j", j=P))
                        adp_ps = ppa.tile([P, CH, 1], F32, space="PSUM")
                        for c in range(CH):
                            nc.tensor.matmul(adp_ps[:, c, :],
                                             lhsT=ohT_sb[:, c, :],
                                             rhs=adst2_all[:, t, :],
                                             start=True, stop=True)
                        alpha = pw.tile([P, CH, 1], F32)
                        nc.vector.tensor_tensor(
                            out=alpha[:], in0=G[:, :, ASRC2:ASRC2 + 1],
                            in1=adp_ps[:], op=A.add)
                        nc.vector.tensor_tensor(
                            out=alpha[:], in0=alpha[:],
                            in1=ae2_sb[:, kb:kb + CH].unsqueeze(2),
                            op=A.add)
                        e2 = pw.tile([P, CH, 1], F32)
                        nc.scalar.activation(out=e2[:], in_=alpha[:],
                                             func=ACT.Exp, scale=0.2)
                        nc.scalar.activation(out=alpha[:], in_=alpha[:],
                                             func=ACT.Exp)
                        p_bf = pw.tile([P, CH, 1], BF16)
                        nc.vector.tensor_tensor(out=p_bf[:], in0=alpha[:],
                                                in1=e2[:], op=A.max)
                        gv = G[:, :, 0:CW]
                        nc.vector.tensor_tensor(
                            out=gv, in0=gv,
                            in1=p_bf[:].to_broadcast([P, CH, CW]),
                            op=A.mult)
                        for c in range(CH):
                            nc.tensor.matmul(
                                acc[:], lhsT=oh_sb[:, c, :],
                                rhs=G[:, c, 0:CW],
                                start=(hh == 0 and c == 0),
                                stop=(hh == 1 and c == CH - 1))
                    # epilogue: o2 = relu(acc/denom + b2) -> bf16, pool matmul
                    dn = pw.tile([P, 1], F32)
                    nc.vector.tensor_scalar(out=dn[:], in0=acc[:, HID:CW],
                                            scalar1=1e-16, scalar2=None,
                                            op0=A.add)
                    rc = pw.tile([P, 1], F32)
                    nc.vector.reciprocal(rc[:], dn[:])
                    o2 = pw.tile([P, HID], F32)
                    nc.vector.tensor_scalar(out=o2[:], in0=acc[:, 0:HID],
                                            scalar1=rc[:, 0:1], scalar2=None,
                                            op0=A.mult)
                    nc.vector.tensor_tensor(out=o2[:], in0=o2[:],
                                            in1=b2_sb[:], op=A.add)
                    o2b = pw.tile([P, HID], BF16)
                    nc.vector.tensor_scalar(out=o2b[:], in0=o2[:],
                                            scalar1=0.0, scalar2=None,
                                            op0=A.max)
                    nc.tensor.matmul(pool_ps[:], lhsT=poolg_sb[:, t, :],
                                     rhs=o2b[:], start=(t == 0),
                                     stop=(t == NT - 1),
                                     skip_group_check=True)

                # ------------- Phase 4: pooled mean + FC --------------------
                fcw_sb = pc.tile([HID, OUT], F32)
                nc.sync.dma_start(fcw_sb[:], fcw_d[:])
                fcb_sb = pc.tile([P, OUT], F32)
                nc.sync.dma_start(fcb_sb[:], fcb_d[:])
                pooled = pc.tile([GPC, HID], F32)
                nc.vector.tensor_scalar(out=pooled[:], in0=pool_ps[:],
                                        scalar1=invc_sb[:, 0:1], scalar2=None,
                                        op0=A.mult)
                pT_ps = ppf.tile([HID, GPC], F32, space="PSUM")
                nc.tensor.transpose(pT_ps[:], pooled[:], identf[:GPC, :GPC])
                pT = pc.tile([HID, GPC], F32)
                nc.vector.tensor_copy(out=pT[:], in_=pT_ps[:])
                fc_ps = ppf.tile([GPC, OUT], F32, space="PSUM")
                nc.tensor.matmul(fc_ps[:], lhsT=pT[:], rhs=fcw_sb[:],
                                 start=True, stop=True)
                res = pc.tile([GPC, OUT], F32)
                nc.vector.tensor_tensor(out=res[:], in0=fc_ps[:],
                                        in1=fcb_sb[:GPC, :], op=A.add)
                nc.sync.dma_start(out_d[:], res[:])

    nc.compile()
    return nc


# ---------------------------------------------------------------------------
# Entry point.
# ---------------------------------------------------------------------------
def run(inputs, cfg, **run_kwargs):
    in_maps, meta = prepare(inputs, cfg)
    nc = build(meta)
    res = run_bass_kernel_spmd(nc, in_maps, core_ids=list(range(NCORES)),
                               **run_kwargs)
    out = np.concatenate([res.results[c]["out"] for c in range(NCORES)],
                         axis=0)
    return np.asarray(out, np.float32), res


def kernel(**inputs) -> np.ndarray:
    out, _ = run(inputs, FULL_CFG)
    return out


# revision 36
# speedup vs baseline: 1.0889x; 1.0889x over previous
"""Trainium2 Bass kernel for a 2-layer edge-featured GAT + mean-pool + FC.

Sharding: 256 graphs split 32-per-core across 8 cores (batch is sorted, so
each core owns a contiguous, graph-aligned node range). Edges live on the
core that owns their destination; per-core node tables are AllGathered
between layers so any core can gather arbitrary source rows.

v2 design (vs the fp32 SWDGE baseline):
- Node tables are bf16. Layer-1 rows are 384 bf16 (768B):
  [h head0(64) | 1 | h1(64) | 1 | h2(64) | 1 | h3(64) | 1 | asrc(4) |
   adst(4) | pad]; the interleaved 1-columns produce softmax denominators
  through the same scatter matmul. Layer-2 rows are 256 bf16 (512B).
- Src-row gathers are SWDGE dma_gather on 4 rotating queues (the Q7
  descriptor generator stalls on ring space with one queue; four queues
  nearly double throughput). Dst-side gathers are gone entirely:
- The edge->dst one-hot matrices (and their transposes) are HOST-PRECOMPUTED
  bf16 constants (the edge structure is input data, not device data).
  oh[128e, 128j] drives the segment-sum scatter matmul; ohT[128j, 128e]
  broadcasts per-dst-node logits to edges via a tiny K=128 matmul
  (adp = ohT^T @ adst_tile).
- p = exp(leaky_relu(asrc+adst+aedge)) is computed per edge (max of two
  exps), multiplied into the message rows including the 1-columns, so one
  matmul per 128-edge chunk accumulates both Sum(p*h) and Sum(p).
  Normalization happens once per node in the epilogue (exactly equivalent
  to the reference's softmax; the max-shift cancels in the ratio).
- Mean-pool one-hot and 1/count are host constants; final FC as in v1.
"""

import sys

sys.path.insert(0, "/opt/trn_rl_repo")

import math
from contextlib import ExitStack

import numpy as np
import ml_dtypes

import concourse.bacc as bacc
import concourse.bass as bass
import concourse.mybir as mybir
import concourse.tile as tile
from concourse.bass_utils import run_bass_kernel_spmd
from concourse.masks import make_identity

P = 128
NCORES = 8
BF = ml_dtypes.bfloat16

FULL_CFG = dict(N=20000, E=640000, FIN=128, HID=64, HEADS=4, NG=256, OUT=32)

F32 = mybir.dt.float32
BF16 = mybir.dt.bfloat16
I16 = mybir.dt.int16

# layer-1 row layout (bf16): 4 x [h(64) | 1] then asrc(4) adst(4) pad -> 384
CW = 65                  # head group width (64 + denom column)
D1R = 4 * CW             # 260
ASRC1, ADST1 = D1R, D1R + 4
ROW1 = 384
# layer-2 row layout (bf16): [h(64) | 1 | asrc(1) | adst(1) | pad] -> 256
ASRC2, ADST2 = CW, CW + 1
ROW2 = 256


# ---------------------------------------------------------------------------
# Host-side preparation: integer index manipulation + array reordering only.
# ---------------------------------------------------------------------------
def prepare(inputs, cfg):
    N, E, FIN, HID, HEADS, NG, OUT = (
        cfg["N"], cfg["E"], cfg["FIN"], cfg["HID"], cfg["HEADS"], cfg["NG"],
        cfg["OUT"],
    )
    GPC = NG // NCORES  # graphs per core

    x = np.asarray(inputs["x"], np.float32)
    ei = np.asarray(inputs["edge_index"], np.int64)
    ea = np.asarray(inputs["edge_attr"], np.float32)
    batch = np.asarray(inputs["batch"], np.int64)
    src, dst = ei[0], ei[1]

    # node ranges per core (graph-aligned; batch is sorted)
    bounds = np.searchsorted(batch, np.arange(NCORES + 1) * GPC)
    node_cnt = np.diff(bounds)
    NT = max(1, math.ceil(node_cnt.max() / P))
    NSLICE = NT * P
    NROWS = NCORES * NSLICE
    assert NROWS < 32768, f"int16 gather index overflow: {NROWS}"

    core_of_node = np.minimum(batch // GPC, NCORES - 1).astype(np.int64)
    rowid = np.empty(N, np.int64)
    for c in range(NCORES):
        ns, ne = bounds[c], bounds[c + 1]
        rowid[ns:ne] = c * NSLICE + np.arange(ne - ns)

    # edges sorted by dst; core blocks are contiguous
    order = np.argsort(dst, kind="stable")
    dsts = dst[order]
    srcs = src[order]
    ws = ea[order, 0]
    ecore = core_of_node[dsts]
    ebounds = np.searchsorted(ecore, np.arange(NCORES + 1))

    # chunks-per-tile: max over all (core, tile), rounded up to even
    cpt_max = 1
    tile_edge_counts = []
    for c in range(NCORES):
        es, ee = ebounds[c], ebounds[c + 1]
        dln = dsts[es:ee] - bounds[c]
        tid = dln // P
        cnts = np.bincount(tid, minlength=NT)
        tile_edge_counts.append(cnts)
        if len(cnts):
            cpt_max = max(cpt_max, math.ceil(cnts.max() / P))
    CPT = cpt_max + (cpt_max % 2)  # even
    CPT = max(CPT, 2)
    CH = CPT // 2
    NCHUNK = NT * CPT

    # per-layer-1/2 attention-edge scalars (tiny float prep, host-replicated)
    q1 = (np.asarray(inputs["We1"], np.float32).reshape(HEADS, HID)
          * np.asarray(inputs["att_edge1"], np.float32)).sum(axis=1)  # [H]
    q2 = float((np.asarray(inputs["We2"], np.float32).reshape(-1)
                * np.asarray(inputs["att_edge2"], np.float32).reshape(-1))
               .sum())

    jj = np.arange(P, dtype=np.int64)

    per_core = []
    for c in range(NCORES):
        ns, ne = bounds[c], bounds[c + 1]
        es, ee = ebounds[c], ebounds[c + 1]
        nloc = ne - ns

        xs = np.zeros((NSLICE, FIN), BF)
        xs[:nloc] = x[ns:ne].astype(BF)

        srcrow = np.zeros((NT, CPT * P), np.int64)
        dstl = np.full((NT, CPT * P), -1, np.int64)
        wv = np.zeros((NT, CPT * P), np.float32)

        dln = dsts[es:ee] - ns
        tid = dln // P
        cnts = tile_edge_counts[c]
        off = np.zeros(NT + 1, np.int64)
        off[1:NT + 1] = np.cumsum(cnts[:NT])
        for t in range(NT):
            k = int(cnts[t]) if t < len(cnts) else 0
            if k == 0:
                continue
            sel = slice(es + int(off[t]), es + int(off[t]) + k)
            srcrow[t, :k] = rowid[srcs[sel]]
            dstl[t, :k] = dln[int(off[t]):int(off[t]) + k] % P
            wv[t, :k] = ws[sel]

        # one-hot constants: eq[k, e, j] = (dstl[k*128+e] == j); fp8e4 (exact
        # 0/1) halves the constant DMA traffic vs bf16
        dstl_f = dstl.reshape(NCHUNK, P)
        eq = (dstl_f[:, :, None] == jj[None, None, :])
        oh_dev = np.ascontiguousarray(
            eq.transpose(1, 0, 2).reshape(P, NCHUNK * P)).astype(
                ml_dtypes.float8_e4m3)
        ohT_dev = np.ascontiguousarray(
            eq.transpose(2, 0, 1).reshape(P, NCHUNK * P)).astype(
                ml_dtypes.float8_e4m3)

        # per-edge attention-edge terms (w_e * q_h); pad slots -> 0
        wflat = wv.reshape(NCHUNK, P)
        ae1 = np.ascontiguousarray(
            (wflat[:, :, None] * q1[None, None, :]).transpose(1, 0, 2)
        ).astype(BF)                                   # [128, NCHUNK, H]
        ae2 = np.ascontiguousarray(
            (wflat * q2).transpose(1, 0)).astype(BF)   # [128, NCHUNK]

        def wrap_idx(arr):  # [NT, CPT*P] -> [128, NT*CPT*8] int16
            blocks = []
            for t in range(NT):
                for h in range(2):
                    ids = arr[t, h * CH * P:(h + 1) * CH * P]
                    a = ids.reshape(CH * 8, 16).T  # [16, CH*8]
                    blocks.append(np.tile(a, (8, 1)))
            return np.ascontiguousarray(
                np.concatenate(blocks, axis=1)).astype(np.int16)

        # per-(tile, half, sub-call) gather counts: sub-calls of 8 and 9
        # chunks; pad slots beyond the count are never gathered (their
        # one-hot columns are zero so the stale data cannot contribute)
        ecnt = np.zeros((NT, 2, 2), np.uint32)
        for t in range(NT):
            k = int(tile_edge_counts[c][t]) if t < len(tile_edge_counts[c]) \
                else 0
            for hh in range(2):
                h = min(max(k - hh * CH * P, 0), CH * P)
                ecnt[t, hh, 0] = min(max(h, 16), 8 * P)
                ecnt[t, hh, 1] = min(max(h - 8 * P, 16), (CH - 8) * P)

        # pooling one-hot + 1/count (host: index data only)
        bl = np.full((NSLICE,), -1, np.int64)
        bl[:nloc] = batch[ns:ne] - c * GPC
        poolg = np.ascontiguousarray(
            (bl.reshape(NT, P)[:, :, None] ==
             np.arange(GPC)[None, None, :]).transpose(1, 0, 2)).astype(BF)
        cnt = np.bincount(bl[:nloc], minlength=GPC).astype(np.float32)
        invc = (1.0 / np.maximum(cnt, 1.0)).reshape(GPC, 1).astype(np.float32)

        per_core.append(dict(
            xs=xs, idxs=wrap_idx(srcrow), oh=oh_dev, ohT=ohT_dev,
            ae1=ae1, ae2=ae2, poolg=poolg, invc=invc,
            ecnt=ecnt.reshape(1, -1).copy(),
        ))

    # weight-side constants (tiny, host-replicated)
    W1 = np.asarray(inputs["W1"], np.float32)            # [FIN, H*HID]
    W2 = np.asarray(inputs["W2"], np.float32)            # [H*HID, HID]
    rep = lambda vv: np.tile(np.asarray(vv, np.float32).reshape(1, -1),
                             (P, 1)).copy()
    consts = dict(
        W1b=W1.astype(BF),
        W2b=np.ascontiguousarray(
            W2.reshape(2, P, HID).transpose(1, 0, 2)).astype(BF),
        as1b=rep(inputs["att_src1"]), ad1b=rep(inputs["att_dst1"]),
        b1b=rep(inputs["b1"]),
        as2b=rep(inputs["att_src2"]), ad2b=rep(inputs["att_dst2"]),
        b2b=rep(inputs["b2"]),
        fcw=np.asarray(inputs["fcW"], np.float32),
        fcbb=rep(inputs["fcb"]),
    )

    in_maps = []
    for c in range(NCORES):
        m = dict(per_core[c])
        m.update(consts)
        in_maps.append(m)

    meta = dict(NT=NT, CPT=CPT, CH=CH, NSLICE=NSLICE, NROWS=NROWS,
                GPC=GPC, NCHUNK=NCHUNK, **cfg)
    return in_maps, meta


# ---------------------------------------------------------------------------
# Device program.
# ---------------------------------------------------------------------------
def build(meta, reps=1, num_devices=NCORES):
    NT, CPT, CH = meta["NT"], meta["CPT"], meta["CH"]
    NSLICE, NROWS, GPC = meta["NSLICE"], meta["NROWS"], meta["GPC"]
    FIN, HID, HEADS, OUT = meta["FIN"], meta["HID"], meta["HEADS"], meta["OUT"]
    NCHUNK = meta["NCHUNK"]
    D1 = HEADS * HID          # 256
    NI = CH * P               # idxs per gather call
    NIc = NI // 16            # idx columns per call
    A = mybir.AluOpType
    ACT = mybir.ActivationFunctionType
    X = mybir.AxisListType.X
    rg = [list(range(NCORES))]

    nc = bacc.Bacc("TRN2", target_bir_lowering=False, debug=False,
                   num_devices=num_devices,
                   dynamic_dma_scratch_size=131072,
                   num_swdge_queues=4)

    def din(name, shape, dtype=F32):
        return nc.dram_tensor(name, list(shape), dtype,
                              kind="ExternalInput").ap()

    F8 = mybir.dt.float8e4
    xs = din("xs", (NSLICE, FIN), BF16)
    idxs_d = din("idxs", (P, NCHUNK * 8), I16)
    oh_d = din("oh", (P, NCHUNK * P), F8)
    ohT_d = din("ohT", (P, NCHUNK * P), F8)
    ae1_d = din("ae1", (P, NCHUNK * HEADS), BF16)
    ae2_d = din("ae2", (P, NCHUNK), BF16)
    poolg_d = din("poolg", (P, NT * GPC), BF16)
    invc_d = din("invc", (GPC, 1))
    W1_d = din("W1b", (FIN, D1), BF16)
    W2_d = din("W2b", (P, 2 * HID), BF16)
    as1_d = din("as1b", (P, D1))
    ad1_d = din("ad1b", (P, D1))
    b1_d = din("b1b", (P, D1))
    as2_d = din("as2b", (P, HID))
    ad2_d = din("ad2b", (P, HID))
    b2_d = din("b2b", (P, HID))
    fcw_d = din("fcw", (HID, OUT))
    fcb_d = din("fcbb", (P, OUT))

    out_d = nc.dram_tensor("out", [GPC, OUT], F32, kind="ExternalOutput").ap()

    gq = [0]  # rotating SWDGE queue
    SUBS = [(0, 8), (8, CH - 8)]  # sub-call (chunk offset, chunk count)

    with tile.TileContext(nc) as tc, ExitStack() as st:
        constp = st.enter_context(tc.tile_pool(name="constp", bufs=1))
        drp = st.enter_context(tc.tile_pool(name="drp", bufs=1, space="DRAM"))

        identf = constp.tile([P, P], F32)
        make_identity(nc, identf[:])
        identb = constp.tile([P, P], BF16)
        make_identity(nc, identb[:])
        ixs_all = constp.tile([P, NCHUNK * 8], I16)
        nc.sync.dma_start(ixs_all[:], idxs_d[:])
        ae1_sb = constp.tile([P, NCHUNK, HEADS], BF16)
        nc.sync.dma_start(ae1_sb[:],
                          ae1_d[:].rearrange("p (k h) -> p k h", h=HEADS))
        ae2_sb = constp.tile([P, NCHUNK], BF16)
        nc.sync.dma_start(ae2_sb[:], ae2_d[:])
        poolg_sb = constp.tile([P, NT, GPC], BF16)
        nc.sync.dma_start(poolg_sb[:],
                          poolg_d[:].rearrange("p (t g) -> p t g", g=GPC))
        invc_sb = constp.tile([GPC, 1], F32)
        nc.sync.dma_start(invc_sb[:], invc_d[:])
        b1_sb = constp.tile([P, D1], F32)
        nc.sync.dma_start(b1_sb[:], b1_d[:])
        def gather_sub(G, full, t, hh, row):
            """Issue the half-tile gather as 2 ring-sized sub-calls so a
            whole call fits a SWDGE ring and the first chunks' matmuls can
            start while the rest still streams."""
            gbase = (t * 2 + hh) * NIc
            for si, (co, cn) in enumerate(SUBS):
                nc.gpsimd.dma_gather(
                    G[:, co:co + cn, :], full[:],
                    ixs_all[:, gbase + co * 8:gbase + (co + cn) * 8],
                    cn * P, cn * P, row, single_packet=False,
                    queue_num=gq[0] % 4)
                gq[0] += 1

        for _rep in range(reps):
            t1loc = drp.tile([NSLICE, ROW1], BF16, name=f"t1loc{_rep}")
            t1full = drp.tile([NROWS, ROW1], BF16, addr_space="Shared",
                              name=f"t1full{_rep}")
            t2loc = drp.tile([NSLICE, ROW2], BF16, name=f"t2loc{_rep}")
            t2full = drp.tile([NROWS, ROW2], BF16, addr_space="Shared",
                              name=f"t2full{_rep}")

            adst1_all = constp.tile([P, NT, HEADS], BF16,
                                    name=f"adst1_{_rep}")
            adst2_all = constp.tile([P, NT, 1], BF16, name=f"adst2_{_rep}")
            out1 = constp.tile([P, NT, D1], BF16, name=f"out1_{_rep}")

            # ------------- Phase 0: h1 = x @ W1, logits, table1 -------------
            with tc.tile_pool(name="ph0", bufs=1) as sp, \
                 tc.tile_pool(name="ph0b", bufs=2) as sp2, \
                 tc.tile_pool(name="ph0p", bufs=2, space="PSUM") as pp:
                w1_sb = sp.tile([P, D1], BF16)
                nc.sync.dma_start(w1_sb[:], W1_d[:])
                as1_sb = sp.tile([P, D1], F32)
                nc.sync.dma_start(as1_sb[:], as1_d[:])
                ad1_sb = sp.tile([P, D1], F32)
                nc.sync.dma_start(ad1_sb[:], ad1_d[:])
                xT_all = sp.tile([P, NSLICE], BF16)
                nc.sync.dma_start_transpose(xT_all[:], xs[:])
                for t in range(NT):
                    h_ps = pp.tile([P, D1], F32, space="PSUM")
                    nc.tensor.matmul(h_ps[:],
                                     lhsT=xT_all[:, t * P:(t + 1) * P],
                                     rhs=w1_sb[:], start=True, stop=True)
                    tmp = sp2.tile([P, D1], F32)
                    red = sp2.tile([P, HEADS], F32)
                    nc.vector.tensor_tensor(out=tmp[:], in0=h_ps[:],
                                            in1=as1_sb[:], op=A.mult)
                    nc.vector.tensor_reduce(
                        out=red[:],
                        in_=tmp[:].rearrange("p (h f) -> p h f", h=HEADS),
                        axis=X, op=A.add)
                    t1t = sp2.tile([P, ROW1], BF16)
                    nc.vector.tensor_copy(out=t1t[:, ASRC1:ASRC1 + HEADS],
                                          in_=red[:])
                    nc.vector.tensor_tensor(out=tmp[:], in0=h_ps[:],
                                            in1=ad1_sb[:], op=A.mult)
                    nc.vector.tensor_reduce(
                        out=red[:],
                        in_=tmp[:].rearrange("p (h f) -> p h f", h=HEADS),
                        axis=X, op=A.add)
                    nc.vector.tensor_copy(out=adst1_all[:, t, :], in_=red[:])
                    nc.vector.tensor_copy(out=t1t[:, ADST1:ADST1 + HEADS],
                                          in_=red[:])
                    hv = t1t[:, 0:D1R].rearrange("p (h f) -> p h f", f=CW)
                    nc.vector.tensor_copy(
                        out=hv[:, :, 0:HID],
                        in_=h_ps[:].rearrange("p (h f) -> p h f", f=HID))
                    nc.vector.memset(hv[:, :, HID:CW], 1.0)
                    nc.vector.memset(t1t[:, ADST1 + HEADS:ROW1], 0.0)
                    nc.sync.dma_start(t1loc[t * P:(t + 1) * P, :], t1t[:])
                nc.gpsimd.collective_compute(
                    "AllGather", A.bypass, replica_groups=rg,
                    ins=[t1loc[:]], outs=[t1full[:]])

            # ------------- Phase 1+2: layer-1 edge phase fused with the
            # per-tile layer-2 GEMM/table build (overlaps AG2 prep) ----------
            with tc.tile_pool(name="p1g", bufs=2) as pg, \
                 tc.tile_pool(name="p1o", bufs=2) as po, \
                 tc.tile_pool(name="p1w", bufs=2) as pw, \
                 tc.tile_pool(name="p1c", bufs=1) as p1c, \
                 tc.tile_pool(name="p1b", bufs=2) as sp2, \
                 tc.tile_pool(name="p1ps", bufs=2, space="PSUM") as pps, \
                 tc.tile_pool(name="p1pa", bufs=2, space="PSUM") as ppa, \
                 tc.tile_pool(name="p1p2", bufs=1, space="PSUM") as pp2:
                w2_sb = p1c.tile([P, 2, HID], BF16)
                nc.sync.dma_start(w2_sb[:],
                                  W2_d[:].rearrange("p (k n) -> p k n", k=2))
                as2_sb = p1c.tile([P, HID], F32)
                nc.sync.dma_start(as2_sb[:], as2_d[:])
                ad2_sb = p1c.tile([P, HID], F32)
                nc.sync.dma_start(ad2_sb[:], ad2_d[:])
                for t in range(NT):
                    acc = pps.tile([P, D1R], F32, space="PSUM")
                    for hh in range(2):
                        kb = t * CPT + hh * CH       # chunk base
                        G = pg.tile([P, CH, ROW1], BF16)
                        gather_sub(G, t1full, t, hh, ROW1)
                        oh_sb = po.tile([P, CH, P], F8)
                        nc.sync.dma_start(
                            oh_sb[:], oh_d[:, kb * P:(kb + CH) * P]
                            .rearrange("p (c j) -> p c j", j=P))
                        ohT_sb = po.tile([P, CH, P], F8)
                        nc.sync.dma_start(
                            ohT_sb[:], ohT_d[:, kb * P:(kb + CH) * P]
                            .rearrange("p (c j) -> p c j", j=P))
                        adp_ps = ppa.tile([P, CH, HEADS], F32, space="PSUM")
                        for c in range(CH):
                            nc.tensor.matmul(adp_ps[:, c, :],
                                             lhsT=ohT_sb[:, c, :],
                                             rhs=adst1_all[:, t, :],
                                             start=True, stop=True)
                        alpha = pw.tile([P, CH, HEADS], F32)
                        nc.vector.tensor_tensor(
                            out=alpha[:], in0=G[:, :, ASRC1:ASRC1 + HEADS],
                            in1=adp_ps[:], op=A.add)
                        nc.vector.tensor_tensor(
                            out=alpha[:], in0=alpha[:],
                            in1=ae1_sb[:, kb:kb + CH, :], op=A.add)
                        e2 = pw.tile([P, CH, HEADS], F32)
                        nc.scalar.activation(out=e2[:], in_=alpha[:],
                                             func=ACT.Exp, scale=0.2)
                        nc.scalar.activation(out=alpha[:], in_=alpha[:],
                                             func=ACT.Exp)
                        p_bf = pw.tile([P, CH, HEADS], BF16)
                        nc.vector.tensor_tensor(out=p_bf[:], in0=alpha[:],
                                                in1=e2[:], op=A.max)
                        gv = G[:, :, 0:D1R].rearrange(
                            "p c (h f) -> p c h f", f=CW)
                        nc.vector.tensor_tensor(
                            out=gv, in0=gv,
                            in1=p_bf[:].unsqueeze(3)
                                .to_broadcast([P, CH, HEADS, CW]),
                            op=A.mult)
                        for c in range(CH):
                            nc.tensor.matmul(
                                acc[:], lhsT=oh_sb[:, c, :],
                                rhs=G[:, c, 0:D1R],
                                start=(hh == 0 and c == 0),
                                stop=(hh == 1 and c == CH - 1))
                    # epilogue: out1 = relu(acc_h / denom_h + b1), fp32 ops
                    # then one contiguous cast (strided bf16 DVE writes
                    # measured pathologically slow on HW)
                    accv = acc[:].rearrange("p (h f) -> p h f", f=CW)
                    dn = pw.tile([P, HEADS], F32)
                    nc.vector.tensor_scalar(out=dn[:], in0=accv[:, :, HID],
                                            scalar1=1e-16, scalar2=None,
                                            op0=A.add)
                    rc = pw.tile([P, HEADS], F32)
                    nc.vector.reciprocal(rc[:], dn[:])
                    o1 = pw.tile([P, HEADS, HID], F32)
                    nc.vector.tensor_tensor(
                        out=o1[:], in0=accv[:, :, 0:HID],
                        in1=rc[:].unsqueeze(2).to_broadcast([P, HEADS, HID]),
                        op=A.mult)
                    nc.vector.tensor_tensor(
                        out=o1[:], in0=o1[:],
                        in1=b1_sb[:].rearrange("p (h f) -> p h f", h=HEADS),
                        op=A.add)
                    nc.scalar.activation(
                        out=out1[:, t, :],
                        in_=o1[:].rearrange("p h f -> p (h f)"),
                        func=ACT.Relu)

                    # layer-2 GEMM + table row for this tile (fused phase 2)
                    h2_ps = pp2.tile([P, HID], F32, space="PSUM")
                    for k in range(2):
                        hT_ps = pp2.tile([P, P], BF16, space="PSUM")
                        nc.tensor.transpose(
                            hT_ps[:], out1[:, t, k * P:(k + 1) * P],
                            identb[:])
                        hT = sp2.tile([P, P], BF16)
                        nc.vector.tensor_copy(out=hT[:], in_=hT_ps[:])
                        nc.tensor.matmul(h2_ps[:], lhsT=hT[:],
                                         rhs=w2_sb[:, k, :],
                                         start=(k == 0), stop=(k == 1))
                    t2t = sp2.tile([P, ROW2], BF16)
                    tmp = sp2.tile([P, HID], F32)
                    red1 = sp2.tile([P, 1], F32)
                    nc.vector.tensor_tensor(out=tmp[:], in0=h2_ps[:],
                                            in1=as2_sb[:], op=A.mult)
                    nc.vector.tensor_reduce(out=red1[:],
                                            in_=tmp[:], axis=X, op=A.add)
                    nc.vector.tensor_copy(out=t2t[:, ASRC2:ASRC2 + 1],
                                          in_=red1[:])
                    nc.vector.tensor_tensor(out=tmp[:], in0=h2_ps[:],
                                            in1=ad2_sb[:], op=A.mult)
                    red2 = sp2.tile([P, 1], F32)
                    nc.vector.tensor_reduce(out=red2[:],
                                            in_=tmp[:], axis=X, op=A.add)
                    nc.vector.tensor_copy(out=adst2_all[:, t, :], in_=red2[:])
                    nc.vector.tensor_copy(out=t2t[:, ADST2:ADST2 + 1],
                                          in_=red2[:])
                    nc.vector.tensor_copy(out=t2t[:, 0:HID], in_=h2_ps[:])
                    nc.vector.memset(t2t[:, HID:HID + 1], 1.0)
                    nc.vector.memset(t2t[:, ADST2 + 1:ROW2], 0.0)
                    nc.sync.dma_start(t2loc[t * P:(t + 1) * P, :], t2t[:])
                nc.gpsimd.collective_compute(
                    "AllGather", A.bypass, replica_groups=rg,
                    ins=[t2loc[:]], outs=[t2full[:]])

            # ------------- Phase 3: layer-2 edge phase + pooling ------------
            with tc.tile_pool(name="p3g", bufs=2) as pg, \
                 tc.tile_pool(name="p3o", bufs=2) as po, \
                 tc.tile_pool(name="p3w", bufs=2) as pw, \
                 tc.tile_pool(name="p3c", bufs=1) as pc, \
                 tc.tile_pool(name="p3ps", bufs=2, space="PSUM") as pps, \
                 tc.tile_pool(name="p3pa", bufs=2, space="PSUM") as ppa, \
                 tc.tile_pool(name="p3f", bufs=1, space="PSUM") as ppf, \
                 tc.tile_pool(name="p3pl", bufs=1, space="PSUM") as ppl:
                b2_sb = pc.tile([P, HID], F32)
                nc.sync.dma_start(b2_sb[:], b2_d[:])
                pool_ps = ppl.tile([GPC, HID], F32, space="PSUM")
                for t in range(NT):
                    acc = pps.tile([P, CW], F32, space="PSUM")
                    for hh in range(2):
                        kb = t * CPT + hh * CH
                        G = pg.tile([P, CH, ROW2], BF16)
                        gather_sub(G, t2full, t, hh, ROW2)
                        oh_sb = po.tile([P, CH, P], F8)
                        nc.sync.dma_start(
                            oh_sb[:], oh_d[:, kb * P:(kb + CH) * P]
                            .rearrange("p (c j) -> p c j", j=P))
                        ohT_sb = po.tile([P, CH, P], F8)
                        nc.sync.dma_start(
                            ohT_sb[:], ohT_d[:, kb * P:(kb + CH) * P]
                            .rearrange("p (c j) -> p c j", j=P))
                        adp_ps = ppa.tile([P, CH, 1], F32, space="PSUM")
                        for c in range(CH):
                            nc.tensor.matmul(adp_ps[:, c, :],
                                             lhsT=ohT_sb[:, c, :],
                                             rhs=adst2_all[:, t, :],
                                             start=True, stop=True)
                        alpha = pw.tile([P, CH, 1], F32)
                        nc.vector.tensor_tensor(
                            out=alpha[:], in0=G[:, :, ASRC2:ASRC2 + 1],
                            in1=adp_ps[:], op=A.add)
                        nc.vector.tensor_tensor(
                            out=alpha[:], in0=alpha[:],
                            in1=ae2_sb[:, kb:kb + CH].unsqueeze(2),
                            op=A.add)
                        e2 = pw.tile([P, CH, 1], F32)
                        nc.scalar.activation(out=e2[:], in_=alpha[:],
                                             func=ACT.Exp, scale=0.2)
                        nc.scalar.activation(out=alpha[:], in_=alpha[:],
                                             func=ACT.Exp)
                        p_bf = pw.tile([P, CH, 1], BF16)
                        nc.vector.tensor_tensor(out=p_bf[:], in0=alpha[:],
                                                in1=e2[:], op=A.max)
                        gv = G[:, :, 0:CW]
                        nc.vector.tensor_tensor(
                            out=gv, in0=gv,
                            in1=p_bf[:].to_broadcast([P, CH, CW]),
                            op=A.mult)
                        for c in range(CH):
                            nc.tensor.matmul(
                                acc[:], lhsT=oh_sb[:, c, :],
                                rhs=G[:, c, 0:CW],
                                start=(hh == 0 and c == 0),
                                stop=(hh == 1 and c == CH - 1))
                    # epilogue: o2 = relu(acc/denom + b2) -> bf16, pool matmul
                    dn = pw.tile([P, 1], F32)
                    nc.vector.tensor_scalar(out=dn[:], in0=acc[:, HID:CW],
                                            scalar1=1e-16, scalar2=None,
                                            op0=A.add)
                    rc = pw.tile([P, 1], F32)
                    nc.vector.reciprocal(rc[:], dn[:])
                    o2 = pw.tile([P, HID], F32)
                    nc.vector.tensor_scalar(out=o2[:], in0=acc[:, 0:HID],
                                            scalar1=rc[:, 0:1], scalar2=None,
                                            op0=A.mult)
                    nc.vector.tensor_tensor(out=o2[:], in0=o2[:],
                                            in1=b2_sb[:], op=A.add)
                    o2b = pw.tile([P, HID], BF16)
                    nc.scalar.activation(out=o2b[:], in_=o2[:],
                                         func=ACT.Relu)
                    nc.tensor.matmul(pool_ps[:], lhsT=poolg_sb[:, t, :],
                                     rhs=o2b[:], start=(t == 0),
                                     stop=(t == NT - 1),
                                     skip_group_check=True)

                # ------------- Phase 4: pooled mean + FC --------------------
                fcw_sb = pc.tile([HID, OUT], F32)
                nc.sync.dma_start(fcw_sb[:], fcw_d[:])
                fcb_sb = pc.tile([P, OUT], F32)
                nc.sync.dma_start(fcb_sb[:], fcb_d[:])
                pooled = pc.tile([GPC, HID], F32)
                nc.vector.tensor_scalar(out=pooled[:], in0=pool_ps[:],
                                        scalar1=invc_sb[:, 0:1], scalar2=None,
                                        op0=A.mult)
                pT_ps = ppf.tile([HID, GPC], F32, space="PSUM")
                nc.tensor.transpose(pT_ps[:], pooled[:], identf[:GPC, :GPC])
                pT = pc.tile([HID, GPC], F32)
                nc.vector.tensor_copy(out=pT[:], in_=pT_ps[:])
                fc_ps = ppf.tile([GPC, OUT], F32, space="PSUM")
                nc.tensor.matmul(fc_ps[:], lhsT=pT[:], rhs=fcw_sb[:],
                                 start=True, stop=True)
                res = pc.tile([GPC, OUT], F32)
                nc.vector.tensor_tensor(out=res[:], in0=fc_ps[:],
                                        in1=fcb_sb[:GPC, :], op=A.add)
                nc.sync.dma_start(out_d[:], res[:])

    nc.compile()
    return nc


# ---------------------------------------------------------------------------
# Entry point.
# ---------------------------------------------------------------------------
def run(inputs, cfg, **run_kwargs):
    in_maps, meta = prepare(inputs, cfg)
    nc = build(meta)
    res = run_bass_kernel_spmd(nc, in_maps, core_ids=list(range(NCORES)),
                               **run_kwargs)
    out = np.concatenate([res.results[c]["out"] for c in range(NCORES)],
                         axis=0)
    return np.asarray(out, np.float32), res


def kernel(**inputs) -> np.ndarray:
    out, _ = run(inputs, FULL_CFG)
    return out


# revision 39
# speedup vs baseline: 1.2094x; 1.1106x over previous
"""Trainium2 Bass kernel for a 2-layer edge-featured GAT + mean-pool + FC.

Sharding: 256 graphs split 32-per-core across 8 cores (batch is sorted, so
each core owns a contiguous, graph-aligned node range). Edges live on the
core that owns their destination; per-core node tables are AllGathered
between layers so any core can gather arbitrary source rows.

v2 design (vs the fp32 SWDGE baseline):
- Node tables are bf16. Layer-1 rows are 384 bf16 (768B):
  [h head0(64) | 1 | h1(64) | 1 | h2(64) | 1 | h3(64) | 1 | asrc(4) |
   adst(4) | pad]; the interleaved 1-columns produce softmax denominators
  through the same scatter matmul. Layer-2 rows are 256 bf16 (512B).
- Src-row gathers are SWDGE dma_gather on 4 rotating queues (the Q7
  descriptor generator stalls on ring space with one queue; four queues
  nearly double throughput). Dst-side gathers are gone entirely:
- The edge->dst one-hot matrices (and their transposes) are HOST-PRECOMPUTED
  bf16 constants (the edge structure is input data, not device data).
  oh[128e, 128j] drives the segment-sum scatter matmul; ohT[128j, 128e]
  broadcasts per-dst-node logits to edges via a tiny K=128 matmul
  (adp = ohT^T @ adst_tile).
- p = exp(leaky_relu(asrc+adst+aedge)) is computed per edge (max of two
  exps), multiplied into the message rows including the 1-columns, so one
  matmul per 128-edge chunk accumulates both Sum(p*h) and Sum(p).
  Normalization happens once per node in the epilogue (exactly equivalent
  to the reference's softmax; the max-shift cancels in the ratio).
- Mean-pool one-hot and 1/count are host constants; final FC as in v1.
"""

import sys

sys.path.insert(0, "/opt/trn_rl_repo")

import math
from contextlib import ExitStack

import numpy as np
import ml_dtypes

import concourse.bacc as bacc
import concourse.bass as bass
import concourse.mybir as mybir
import concourse.tile as tile
from concourse.bass_utils import run_bass_kernel_spmd
from concourse.masks import make_identity

P = 128
NCORES = 8
BF = ml_dtypes.bfloat16

FULL_CFG = dict(N=20000, E=640000, FIN=128, HID=64, HEADS=4, NG=256, OUT=32)

F32 = mybir.dt.float32
BF16 = mybir.dt.bfloat16
I16 = mybir.dt.int16

# layer-1 row layout (bf16): 4 x [h(64) | 1] then asrc(4) adst(4) pad -> 384
CW = 65                  # head group width (64 + denom column)
D1R = 4 * CW             # 260
ASRC1, ADST1 = D1R, D1R + 4
ROW1 = 384
# layer-2 row layout (bf16): [h(64) | 1 | asrc(1) | adst(1) | pad] -> 128
ASRC2, ADST2 = CW, CW + 1
ROW2 = 128


# ---------------------------------------------------------------------------
# Host-side preparation: integer index manipulation + array reordering only.
# ---------------------------------------------------------------------------
def prepare(inputs, cfg):
    N, E, FIN, HID, HEADS, NG, OUT = (
        cfg["N"], cfg["E"], cfg["FIN"], cfg["HID"], cfg["HEADS"], cfg["NG"],
        cfg["OUT"],
    )
    GPC = NG // NCORES  # graphs per core

    x = np.asarray(inputs["x"], np.float32)
    ei = np.asarray(inputs["edge_index"], np.int64)
    ea = np.asarray(inputs["edge_attr"], np.float32)
    batch = np.asarray(inputs["batch"], np.int64)
    src, dst = ei[0], ei[1]

    # node ranges per core (graph-aligned; batch is sorted)
    bounds = np.searchsorted(batch, np.arange(NCORES + 1) * GPC)
    node_cnt = np.diff(bounds)
    NT = max(1, math.ceil(node_cnt.max() / P))
    NSLICE = NT * P
    NROWS = NCORES * NSLICE
    assert NROWS < 32768, f"int16 gather index overflow: {NROWS}"

    core_of_node = np.minimum(batch // GPC, NCORES - 1).astype(np.int64)
    rowid = np.empty(N, np.int64)
    for c in range(NCORES):
        ns, ne = bounds[c], bounds[c + 1]
        rowid[ns:ne] = c * NSLICE + np.arange(ne - ns)

    # edges sorted by dst; core blocks are contiguous
    order = np.argsort(dst, kind="stable")
    dsts = dst[order]
    srcs = src[order]
    ws = ea[order, 0]
    ecore = core_of_node[dsts]
    ebounds = np.searchsorted(ecore, np.arange(NCORES + 1))

    # chunks-per-tile: max over all (core, tile), rounded up to even
    cpt_max = 1
    tile_edge_counts = []
    for c in range(NCORES):
        es, ee = ebounds[c], ebounds[c + 1]
        dln = dsts[es:ee] - bounds[c]
        tid = dln // P
        cnts = np.bincount(tid, minlength=NT)
        tile_edge_counts.append(cnts)
        if len(cnts):
            cpt_max = max(cpt_max, math.ceil(cnts.max() / P))
    CPT = cpt_max + (cpt_max % 2)  # even
    CPT = max(CPT, 2)
    CH = CPT // 2
    NCHUNK = NT * CPT

    # per-layer-1/2 attention-edge scalars (tiny float prep, host-replicated)
    q1 = (np.asarray(inputs["We1"], np.float32).reshape(HEADS, HID)
          * np.asarray(inputs["att_edge1"], np.float32)).sum(axis=1)  # [H]
    q2 = float((np.asarray(inputs["We2"], np.float32).reshape(-1)
                * np.asarray(inputs["att_edge2"], np.float32).reshape(-1))
               .sum())

    jj = np.arange(P, dtype=np.int64)

    per_core = []
    for c in range(NCORES):
        ns, ne = bounds[c], bounds[c + 1]
        es, ee = ebounds[c], ebounds[c + 1]
        nloc = ne - ns

        xs = np.zeros((NSLICE, FIN), BF)
        xs[:nloc] = x[ns:ne].astype(BF)

        srcrow = np.zeros((NT, CPT * P), np.int64)
        dstl = np.full((NT, CPT * P), -1, np.int64)
        wv = np.zeros((NT, CPT * P), np.float32)

        dln = dsts[es:ee] - ns
        tid = dln // P
        cnts = tile_edge_counts[c]
        off = np.zeros(NT + 1, np.int64)
        off[1:NT + 1] = np.cumsum(cnts[:NT])
        for t in range(NT):
            k = int(cnts[t]) if t < len(cnts) else 0
            if k == 0:
                continue
            sel = slice(es + int(off[t]), es + int(off[t]) + k)
            srcrow[t, :k] = rowid[srcs[sel]]
            dstl[t, :k] = dln[int(off[t]):int(off[t]) + k] % P
            wv[t, :k] = ws[sel]

        # one-hot constants: eq[k, e, j] = (dstl[k*128+e] == j); fp8e4 (exact
        # 0/1) halves the constant DMA traffic vs bf16
        dstl_f = dstl.reshape(NCHUNK, P)
        eq = (dstl_f[:, :, None] == jj[None, None, :])
        oh_dev = np.ascontiguousarray(
            eq.transpose(1, 0, 2).reshape(P, NCHUNK * P)).astype(
                ml_dtypes.float8_e4m3)
        ohT_dev = np.ascontiguousarray(
            eq.transpose(2, 0, 1).reshape(P, NCHUNK * P)).astype(
                ml_dtypes.float8_e4m3)

        # per-edge attention-edge terms (w_e * q_h); pad slots -> 0
        wflat = wv.reshape(NCHUNK, P)
        ae1 = np.ascontiguousarray(
            (wflat[:, :, None] * q1[None, None, :]).transpose(1, 0, 2)
        ).astype(BF)                                   # [128, NCHUNK, H]
        ae2 = np.ascontiguousarray(
            (wflat * q2).transpose(1, 0)).astype(BF)   # [128, NCHUNK]

        def wrap_idx(arr):  # [NT, CPT*P] -> [128, NT*CPT*8] int16
            blocks = []
            for t in range(NT):
                for h in range(2):
                    ids = arr[t, h * CH * P:(h + 1) * CH * P]
                    a = ids.reshape(CH * 8, 16).T  # [16, CH*8]
                    blocks.append(np.tile(a, (8, 1)))
            return np.ascontiguousarray(
                np.concatenate(blocks, axis=1)).astype(np.int16)

        # per-(tile, half, sub-call) gather counts: sub-calls of 8 and 9
        # chunks; pad slots beyond the count are never gathered (their
        # one-hot columns are zero so the stale data cannot contribute)
        ecnt = np.zeros((NT, 2, 2), np.uint32)
        for t in range(NT):
            k = int(tile_edge_counts[c][t]) if t < len(tile_edge_counts[c]) \
                else 0
            for hh in range(2):
                h = min(max(k - hh * CH * P, 0), CH * P)
                ecnt[t, hh, 0] = min(max(h, 16), 8 * P)
                ecnt[t, hh, 1] = min(max(h - 8 * P, 16), (CH - 8) * P)

        # pooling one-hot + 1/count (host: index data only)
        bl = np.full((NSLICE,), -1, np.int64)
        bl[:nloc] = batch[ns:ne] - c * GPC
        poolg = np.ascontiguousarray(
            (bl.reshape(NT, P)[:, :, None] ==
             np.arange(GPC)[None, None, :]).transpose(1, 0, 2)).astype(BF)
        cnt = np.bincount(bl[:nloc], minlength=GPC).astype(np.float32)
        invc = (1.0 / np.maximum(cnt, 1.0)).reshape(GPC, 1).astype(np.float32)

        per_core.append(dict(
            xs=xs, idxs=wrap_idx(srcrow), oh=oh_dev, ohT=ohT_dev,
            ae1=ae1, ae2=ae2, poolg=poolg, invc=invc,
            ecnt=ecnt.reshape(1, -1).copy(),
        ))

    # weight-side constants (tiny, host-replicated)
    W1 = np.asarray(inputs["W1"], np.float32)            # [FIN, H*HID]
    W2 = np.asarray(inputs["W2"], np.float32)            # [H*HID, HID]
    rep = lambda vv: np.tile(np.asarray(vv, np.float32).reshape(1, -1),
                             (P, 1)).copy()
    consts = dict(
        W1b=W1.astype(BF),
        W2b=np.ascontiguousarray(
            W2.reshape(2, P, HID).transpose(1, 0, 2)).astype(BF),
        as1b=rep(inputs["att_src1"]), ad1b=rep(inputs["att_dst1"]),
        b1b=rep(inputs["b1"]),
        as2b=rep(inputs["att_src2"]), ad2b=rep(inputs["att_dst2"]),
        b2b=rep(inputs["b2"]),
        fcw=np.asarray(inputs["fcW"], np.float32),
        fcbb=rep(inputs["fcb"]),
    )

    in_maps = []
    for c in range(NCORES):
        m = dict(per_core[c])
        m.update(consts)
        in_maps.append(m)

    meta = dict(NT=NT, CPT=CPT, CH=CH, NSLICE=NSLICE, NROWS=NROWS,
                GPC=GPC, NCHUNK=NCHUNK, **cfg)
    return in_maps, meta


# ---------------------------------------------------------------------------
# Device program.
# ---------------------------------------------------------------------------
def build(meta, reps=1, num_devices=NCORES):
    NT, CPT, CH = meta["NT"], meta["CPT"], meta["CH"]
    NSLICE, NROWS, GPC = meta["NSLICE"], meta["NROWS"], meta["GPC"]
    FIN, HID, HEADS, OUT = meta["FIN"], meta["HID"], meta["HEADS"], meta["OUT"]
    NCHUNK = meta["NCHUNK"]
    D1 = HEADS * HID          # 256
    NI = CH * P               # idxs per gather call
    NIc = NI // 16            # idx columns per call
    A = mybir.AluOpType
    ACT = mybir.ActivationFunctionType
    X = mybir.AxisListType.X
    rg = [list(range(NCORES))]

    nc = bacc.Bacc("TRN2", target_bir_lowering=False, debug=False,
                   num_devices=num_devices,
                   dynamic_dma_scratch_size=131072,
                   num_swdge_queues=4)

    def din(name, shape, dtype=F32):
        return nc.dram_tensor(name, list(shape), dtype,
                              kind="ExternalInput").ap()

    F8 = mybir.dt.float8e4
    xs = din("xs", (NSLICE, FIN), BF16)
    idxs_d = din("idxs", (P, NCHUNK * 8), I16)
    oh_d = din("oh", (P, NCHUNK * P), F8)
    ohT_d = din("ohT", (P, NCHUNK * P), F8)
    ae1_d = din("ae1", (P, NCHUNK * HEADS), BF16)
    ae2_d = din("ae2", (P, NCHUNK), BF16)
    poolg_d = din("poolg", (P, NT * GPC), BF16)
    invc_d = din("invc", (GPC, 1))
    W1_d = din("W1b", (FIN, D1), BF16)
    W2_d = din("W2b", (P, 2 * HID), BF16)
    as1_d = din("as1b", (P, D1))
    ad1_d = din("ad1b", (P, D1))
    b1_d = din("b1b", (P, D1))
    as2_d = din("as2b", (P, HID))
    ad2_d = din("ad2b", (P, HID))
    b2_d = din("b2b", (P, HID))
    fcw_d = din("fcw", (HID, OUT))
    fcb_d = din("fcbb", (P, OUT))

    out_d = nc.dram_tensor("out", [GPC, OUT], F32, kind="ExternalOutput").ap()

    gq = [0]  # rotating SWDGE queue
    SUBS = [(0, 8), (8, CH - 8)]  # sub-call (chunk offset, chunk count)

    with tile.TileContext(nc) as tc, ExitStack() as st:
        constp = st.enter_context(tc.tile_pool(name="constp", bufs=1))
        drp = st.enter_context(tc.tile_pool(name="drp", bufs=1, space="DRAM"))

        identf = constp.tile([P, P], F32)
        make_identity(nc, identf[:])
        identb = constp.tile([P, P], BF16)
        make_identity(nc, identb[:])
        ixs_all = constp.tile([P, NCHUNK * 8], I16)
        nc.sync.dma_start(ixs_all[:], idxs_d[:])
        ae1_sb = constp.tile([P, NCHUNK, HEADS], BF16)
        nc.sync.dma_start(ae1_sb[:],
                          ae1_d[:].rearrange("p (k h) -> p k h", h=HEADS))
        ae2_sb = constp.tile([P, NCHUNK], BF16)
        nc.sync.dma_start(ae2_sb[:], ae2_d[:])
        poolg_sb = constp.tile([P, NT, GPC], BF16)
        nc.sync.dma_start(poolg_sb[:],
                          poolg_d[:].rearrange("p (t g) -> p t g", g=GPC))
        invc_sb = constp.tile([GPC, 1], F32)
        nc.sync.dma_start(invc_sb[:], invc_d[:])
        b1_sb = constp.tile([P, D1], F32)
        nc.sync.dma_start(b1_sb[:], b1_d[:])
        def gather_sub(G, full, t, hh, row):
            """Issue the half-tile gather as 2 ring-sized sub-calls so a
            whole call fits a SWDGE ring and the first chunks' matmuls can
            start while the rest still streams."""
            gbase = (t * 2 + hh) * NIc
            for si, (co, cn) in enumerate(SUBS):
                nc.gpsimd.dma_gather(
                    G[:, co:co + cn, :], full[:],
                    ixs_all[:, gbase + co * 8:gbase + (co + cn) * 8],
                    cn * P, cn * P, row, single_packet=False,
                    queue_num=gq[0] % 4)
                gq[0] += 1

        for _rep in range(reps):
            t1loc = drp.tile([NSLICE, ROW1], BF16, name=f"t1loc{_rep}")
            t1full = drp.tile([NROWS, ROW1], BF16, addr_space="Shared",
                              name=f"t1full{_rep}")
            t2loc = drp.tile([NSLICE, ROW2], BF16, name=f"t2loc{_rep}")
            t2full = drp.tile([NROWS, ROW2], BF16, addr_space="Shared",
                              name=f"t2full{_rep}")

            adst1_all = constp.tile([P, NT, HEADS], BF16,
                                    name=f"adst1_{_rep}")
            adst2_all = constp.tile([P, NT, 1], BF16, name=f"adst2_{_rep}")
            out1 = constp.tile([P, NT, D1], BF16, name=f"out1_{_rep}")

            # ------------- Phase 0: h1 = x @ W1, logits, table1 -------------
            with tc.tile_pool(name="ph0", bufs=1) as sp, \
                 tc.tile_pool(name="ph0b", bufs=2) as sp2, \
                 tc.tile_pool(name="ph0p", bufs=2, space="PSUM") as pp:
                w1_sb = sp.tile([P, D1], BF16)
                nc.sync.dma_start(w1_sb[:], W1_d[:])
                as1_sb = sp.tile([P, D1], F32)
                nc.sync.dma_start(as1_sb[:], as1_d[:])
                ad1_sb = sp.tile([P, D1], F32)
                nc.sync.dma_start(ad1_sb[:], ad1_d[:])
                xT_all = sp.tile([P, NSLICE], BF16)
                nc.sync.dma_start_transpose(xT_all[:], xs[:])
                for t in range(NT):
                    h_ps = pp.tile([P, D1], F32, space="PSUM")
                    nc.tensor.matmul(h_ps[:],
                                     lhsT=xT_all[:, t * P:(t + 1) * P],
                                     rhs=w1_sb[:], start=True, stop=True)
                    tmp = sp2.tile([P, D1], F32)
                    red = sp2.tile([P, HEADS], F32)
                    nc.vector.tensor_tensor(out=tmp[:], in0=h_ps[:],
                                            in1=as1_sb[:], op=A.mult)
                    nc.vector.tensor_reduce(
                        out=red[:],
                        in_=tmp[:].rearrange("p (h f) -> p h f", h=HEADS),
                        axis=X, op=A.add)
                    t1t = sp2.tile([P, ROW1], BF16)
                    nc.vector.tensor_copy(out=t1t[:, ASRC1:ASRC1 + HEADS],
                                          in_=red[:])
                    nc.vector.tensor_tensor(out=tmp[:], in0=h_ps[:],
                                            in1=ad1_sb[:], op=A.mult)
                    nc.vector.tensor_reduce(
                        out=red[:],
                        in_=tmp[:].rearrange("p (h f) -> p h f", h=HEADS),
                        axis=X, op=A.add)
                    nc.vector.tensor_copy(out=adst1_all[:, t, :], in_=red[:])
                    nc.vector.tensor_copy(out=t1t[:, ADST1:ADST1 + HEADS],
                                          in_=red[:])
                    hv = t1t[:, 0:D1R].rearrange("p (h f) -> p h f", f=CW)
                    nc.vector.tensor_copy(
                        out=hv[:, :, 0:HID],
                        in_=h_ps[:].rearrange("p (h f) -> p h f", f=HID))
                    nc.vector.memset(hv[:, :, HID:CW], 1.0)
                    nc.vector.memset(t1t[:, ADST1 + HEADS:ROW1], 0.0)
                    nc.sync.dma_start(t1loc[t * P:(t + 1) * P, :], t1t[:])
                nc.gpsimd.collective_compute(
                    "AllGather", A.bypass, replica_groups=rg,
                    ins=[t1loc[:]], outs=[t1full[:]])

            # ------------- Phase 1+2: layer-1 edge phase fused with the
            # per-tile layer-2 GEMM/table build (overlaps AG2 prep) ----------
            with tc.tile_pool(name="p1g", bufs=3) as pg, \
                 tc.tile_pool(name="p1o", bufs=2) as po, \
                 tc.tile_pool(name="p1w", bufs=2) as pw, \
                 tc.tile_pool(name="p1c", bufs=1) as p1c, \
                 tc.tile_pool(name="p1b", bufs=2) as sp2, \
                 tc.tile_pool(name="p1ps", bufs=2, space="PSUM") as pps, \
                 tc.tile_pool(name="p1pa", bufs=2, space="PSUM") as ppa, \
                 tc.tile_pool(name="p1p2", bufs=1, space="PSUM") as pp2:
                w2_sb = p1c.tile([P, 2, HID], BF16)
                nc.sync.dma_start(w2_sb[:],
                                  W2_d[:].rearrange("p (k n) -> p k n", k=2))
                as2_sb = p1c.tile([P, HID], F32)
                nc.sync.dma_start(as2_sb[:], as2_d[:])
                ad2_sb = p1c.tile([P, HID], F32)
                nc.sync.dma_start(ad2_sb[:], ad2_d[:])
                for t in range(NT):
                    acc = pps.tile([P, D1R], F32, space="PSUM")
                    for hh in range(2):
                        kb = t * CPT + hh * CH       # chunk base
                        G = pg.tile([P, CH, ROW1], BF16)
                        gather_sub(G, t1full, t, hh, ROW1)
                        oh_sb = po.tile([P, CH, P], F8)
                        nc.sync.dma_start(
                            oh_sb[:], oh_d[:, kb * P:(kb + CH) * P]
                            .rearrange("p (c j) -> p c j", j=P))
                        ohT_sb = po.tile([P, CH, P], F8)
                        nc.sync.dma_start(
                            ohT_sb[:], ohT_d[:, kb * P:(kb + CH) * P]
                            .rearrange("p (c j) -> p c j", j=P))
                        adp_ps = ppa.tile([P, CH, HEADS], F32, space="PSUM")
                        for c in range(CH):
                            nc.tensor.matmul(adp_ps[:, c, :],
                                             lhsT=ohT_sb[:, c, :],
                                             rhs=adst1_all[:, t, :],
                                             start=True, stop=True)
                        alpha = pw.tile([P, CH, HEADS], F32)
                        nc.vector.tensor_tensor(
                            out=alpha[:], in0=G[:, :, ASRC1:ASRC1 + HEADS],
                            in1=adp_ps[:], op=A.add)
                        nc.vector.tensor_tensor(
                            out=alpha[:], in0=alpha[:],
                            in1=ae1_sb[:, kb:kb + CH, :], op=A.add)
                        e2 = pw.tile([P, CH, HEADS], F32)
                        nc.scalar.activation(out=e2[:], in_=alpha[:],
                                             func=ACT.Exp, scale=0.2)
                        nc.scalar.activation(out=alpha[:], in_=alpha[:],
                                             func=ACT.Exp)
                        p_bf = pw.tile([P, CH, HEADS], BF16)
                        nc.vector.tensor_tensor(out=p_bf[:], in0=alpha[:],
                                                in1=e2[:], op=A.max)
                        gv = G[:, :, 0:D1R].rearrange(
                            "p c (h f) -> p c h f", f=CW)
                        nc.vector.tensor_tensor(
                            out=gv, in0=gv,
                            in1=p_bf[:].unsqueeze(3)
                                .to_broadcast([P, CH, HEADS, CW]),
                            op=A.mult)
                        for c in range(CH):
                            nc.tensor.matmul(
                                acc[:], lhsT=oh_sb[:, c, :],
                                rhs=G[:, c, 0:D1R],
                                start=(hh == 0 and c == 0),
                                stop=(hh == 1 and c == CH - 1))
                    # epilogue: out1 = relu(acc_h / denom_h + b1), fp32 ops
                    # then one contiguous cast (strided bf16 DVE writes
                    # measured pathologically slow on HW)
                    accv = acc[:].rearrange("p (h f) -> p h f", f=CW)
                    dn = pw.tile([P, HEADS], F32)
                    nc.vector.tensor_scalar(out=dn[:], in0=accv[:, :, HID],
                                            scalar1=1e-16, scalar2=None,
                                            op0=A.add)
                    rc = pw.tile([P, HEADS], F32)
                    nc.vector.reciprocal(rc[:], dn[:])
                    o1 = pw.tile([P, HEADS, HID], F32)
                    nc.vector.tensor_tensor(
                        out=o1[:], in0=accv[:, :, 0:HID],
                        in1=rc[:].unsqueeze(2).to_broadcast([P, HEADS, HID]),
                        op=A.mult)
                    nc.vector.tensor_tensor(
                        out=o1[:], in0=o1[:],
                        in1=b1_sb[:].rearrange("p (h f) -> p h f", h=HEADS),
                        op=A.add)
                    nc.scalar.activation(
                        out=out1[:, t, :],
                        in_=o1[:].rearrange("p h f -> p (h f)"),
                        func=ACT.Relu)

                    # layer-2 GEMM + table row for this tile (fused phase 2)
                    h2_ps = pp2.tile([P, HID], F32, space="PSUM")
                    for k in range(2):
                        hT_ps = pp2.tile([P, P], BF16, space="PSUM")
                        nc.tensor.transpose(
                            hT_ps[:], out1[:, t, k * P:(k + 1) * P],
                            identb[:])
                        hT = sp2.tile([P, P], BF16)
                        nc.vector.tensor_copy(out=hT[:], in_=hT_ps[:])
                        nc.tensor.matmul(h2_ps[:], lhsT=hT[:],
                                         rhs=w2_sb[:, k, :],
                                         start=(k == 0), stop=(k == 1))
                    t2t = sp2.tile([P, ROW2], BF16)
                    tmp = sp2.tile([P, HID], F32)
                    red1 = sp2.tile([P, 1], F32)
                    nc.vector.tensor_tensor(out=tmp[:], in0=h2_ps[:],
                                            in1=as2_sb[:], op=A.mult)
                    nc.vector.tensor_reduce(out=red1[:],
                                            in_=tmp[:], axis=X, op=A.add)
                    nc.vector.tensor_copy(out=t2t[:, ASRC2:ASRC2 + 1],
                                          in_=red1[:])
                    nc.vector.tensor_tensor(out=tmp[:], in0=h2_ps[:],
                                            in1=ad2_sb[:], op=A.mult)
                    red2 = sp2.tile([P, 1], F32)
                    nc.vector.tensor_reduce(out=red2[:],
                                            in_=tmp[:], axis=X, op=A.add)
                    nc.vector.tensor_copy(out=adst2_all[:, t, :], in_=red2[:])
                    nc.vector.tensor_copy(out=t2t[:, ADST2:ADST2 + 1],
                                          in_=red2[:])
                    nc.vector.tensor_copy(out=t2t[:, 0:HID], in_=h2_ps[:])
                    nc.vector.memset(t2t[:, HID:HID + 1], 1.0)
                    nc.vector.memset(t2t[:, ADST2 + 1:ROW2], 0.0)
                    nc.sync.dma_start(t2loc[t * P:(t + 1) * P, :], t2t[:])
                nc.gpsimd.collective_compute(
                    "AllGather", A.bypass, replica_groups=rg,
                    ins=[t2loc[:]], outs=[t2full[:]])

            # ------------- Phase 3: layer-2 edge phase + pooling ------------
            with tc.tile_pool(name="p3g", bufs=3) as pg, \
                 tc.tile_pool(name="p3o", bufs=2) as po, \
                 tc.tile_pool(name="p3w", bufs=2) as pw, \
                 tc.tile_pool(name="p3c", bufs=1) as pc, \
                 tc.tile_pool(name="p3ps", bufs=2, space="PSUM") as pps, \
                 tc.tile_pool(name="p3pa", bufs=2, space="PSUM") as ppa, \
                 tc.tile_pool(name="p3f", bufs=1, space="PSUM") as ppf, \
                 tc.tile_pool(name="p3pl", bufs=1, space="PSUM") as ppl:
                b2_sb = pc.tile([P, HID], F32)
                nc.sync.dma_start(b2_sb[:], b2_d[:])
                pool_ps = ppl.tile([GPC, HID], F32, space="PSUM")
                for t in range(NT):
                    acc = pps.tile([P, CW], F32, space="PSUM")
                    for hh in range(2):
                        kb = t * CPT + hh * CH
                        G = pg.tile([P, CH, ROW2], BF16)
                        gather_sub(G, t2full, t, hh, ROW2)
                        oh_sb = po.tile([P, CH, P], F8)
                        nc.sync.dma_start(
                            oh_sb[:], oh_d[:, kb * P:(kb + CH) * P]
                            .rearrange("p (c j) -> p c j", j=P))
                        ohT_sb = po.tile([P, CH, P], F8)
                        nc.sync.dma_start(
                            ohT_sb[:], ohT_d[:, kb * P:(kb + CH) * P]
                            .rearrange("p (c j) -> p c j", j=P))
                        adp_ps = ppa.tile([P, CH, 1], F32, space="PSUM")
                        for c in range(CH):
                            nc.tensor.matmul(adp_ps[:, c, :],
                                             lhsT=ohT_sb[:, c, :],
                                             rhs=adst2_all[:, t, :],
                                             start=True, stop=True)
                        alpha = pw.tile([P, CH, 1], F32)
                        nc.vector.tensor_tensor(
                            out=alpha[:], in0=G[:, :, ASRC2:ASRC2 + 1],
                            in1=adp_ps[:], op=A.add)
                        nc.vector.tensor_tensor(
                            out=alpha[:], in0=alpha[:],
                            in1=ae2_sb[:, kb:kb + CH].unsqueeze(2),
                            op=A.add)
                        e2 = pw.tile([P, CH, 1], F32)
                        nc.scalar.activation(out=e2[:], in_=alpha[:],
                                             func=ACT.Exp, scale=0.2)
                        nc.scalar.activation(out=alpha[:], in_=alpha[:],
                                             func=ACT.Exp)
                        p_bf = pw.tile([P, CH, 1], BF16)
                        nc.vector.tensor_tensor(out=p_bf[:], in0=alpha[:],
                                                in1=e2[:], op=A.max)
                        gv = G[:, :, 0:CW]
                        nc.vector.tensor_tensor(
                            out=gv, in0=gv,
                            in1=p_bf[:].to_broadcast([P, CH, CW]),
                            op=A.mult)
                        for c in range(CH):
                            nc.tensor.matmul(
                                acc[:], lhsT=oh_sb[:, c, :],
                                rhs=G[:, c, 0:CW],
                                start=(hh == 0 and c == 0),
                                stop=(hh == 1 and c == CH - 1))
                    # epilogue: o2 = relu(acc/denom + b2) -> bf16, pool matmul
                    dn = pw.tile([P, 1], F32)
                    nc.vector.tensor_scalar(out=dn[:], in0=acc[:, HID:CW],
                                            scalar1=1e-16, scalar2=None,
                                            op0=A.add)
                    rc = pw.tile([P, 1], F32)
                    nc.vector.reciprocal(rc[:], dn[:])
                    o2 = pw.tile([P, HID], F32)
                    nc.vector.tensor_scalar(out=o2[:], in0=acc[:, 0:HID],
                                            scalar1=rc[:, 0:1], scalar2=None,
                                            op0=A.mult)
                    nc.vector.tensor_tensor(out=o2[:], in0=o2[:],
                                            in1=b2_sb[:], op=A.add)
                    o2b = pw.tile([P, HID], BF16)
                    nc.scalar.activation(out=o2b[:], in_=o2[:],
                                         func=ACT.Relu)
                    nc.tensor.matmul(pool_ps[:], lhsT=poolg_sb[:, t, :],
                                     rhs=o2b[:], start=(t == 0),
                                     stop=(t == NT - 1),
                                     skip_group_check=True)

                # ------------- Phase 4: pooled mean + FC --------------------
                fcw_sb = pc.tile([HID, OUT], F32)
                nc.sync.dma_start(fcw_sb[:], fcw_d[:])
                fcb_sb = pc.tile([P, OUT], F32)
                nc.sync.dma_start(fcb_sb[:], fcb_d[:])
                pooled = pc.tile([GPC, HID], F32)
                nc.vector.tensor_scalar(out=pooled[:], in0=pool_ps[:],
                                        scalar1=invc_sb[:, 0:1], scalar2=None,
                                        op0=A.mult)
                pT_ps = ppf.tile([HID, GPC], F32, space="PSUM")
                nc.tensor.transpose(pT_ps[:], pooled[:], identf[:GPC, :GPC])
                pT = pc.tile([HID, GPC], F32)
                nc.vector.tensor_copy(out=pT[:], in_=pT_ps[:])
                fc_ps = ppf.tile([GPC, OUT], F32, space="PSUM")
                nc.tensor.matmul(fc_ps[:], lhsT=pT[:], rhs=fcw_sb[:],
                                 start=True, stop=True)
                res = pc.tile([GPC, OUT], F32)
                nc.vector.tensor_tensor(out=res[:], in0=fc_ps[:],
                                        in1=fcb_sb[:GPC, :], op=A.add)
                nc.sync.dma_start(out_d[:], res[:])

    nc.compile()
    return nc


# ---------------------------------------------------------------------------
# Entry point.
# ---------------------------------------------------------------------------
def run(inputs, cfg, **run_kwargs):
    in_maps, meta = prepare(inputs, cfg)
    nc = build(meta)
    res = run_bass_kernel_spmd(nc, in_maps, core_ids=list(range(NCORES)),
                               **run_kwargs)
    out = np.concatenate([res.results[c]["out"] for c in range(NCORES)],
                         axis=0)
    return np.asarray(out, np.float32), res


def kernel(**inputs) -> np.ndarray:
    out, _ = run(inputs, FULL_CFG)
    return out


# revision 43
# speedup vs baseline: 1.2364x; 1.0223x over previous
"""Trainium2 Bass kernel for a 2-layer edge-featured GAT + mean-pool + FC.

Sharding: 256 graphs split 32-per-core across 8 cores (batch is sorted, so
each core owns a contiguous, graph-aligned node range). Edges live on the
core that owns their destination; per-core node tables are AllGathered
between layers so any core can gather arbitrary source rows.

v2 design (vs the fp32 SWDGE baseline):
- Node tables are bf16. Layer-1 rows are 384 bf16 (768B):
  [h head0(64) | 1 | h1(64) | 1 | h2(64) | 1 | h3(64) | 1 | asrc(4) |
   adst(4) | pad]; the interleaved 1-columns produce softmax denominators
  through the same scatter matmul. Layer-2 rows are 256 bf16 (512B).
- Src-row gathers are SWDGE dma_gather on 4 rotating queues (the Q7
  descriptor generator stalls on ring space with one queue; four queues
  nearly double throughput). Dst-side gathers are gone entirely:
- The edge->dst one-hot matrices (and their transposes) are HOST-PRECOMPUTED
  bf16 constants (the edge structure is input data, not device data).
  oh[128e, 128j] drives the segment-sum scatter matmul; ohT[128j, 128e]
  broadcasts per-dst-node logits to edges via a tiny K=128 matmul
  (adp = ohT^T @ adst_tile).
- p = exp(leaky_relu(asrc+adst+aedge)) is computed per edge (max of two
  exps), multiplied into the message rows including the 1-columns, so one
  matmul per 128-edge chunk accumulates both Sum(p*h) and Sum(p).
  Normalization happens once per node in the epilogue (exactly equivalent
  to the reference's softmax; the max-shift cancels in the ratio).
- Mean-pool one-hot and 1/count are host constants; final FC as in v1.
"""

import sys

sys.path.insert(0, "/opt/trn_rl_repo")

import math
from contextlib import ExitStack

import numpy as np
import ml_dtypes

import concourse.bacc as bacc
import concourse.bass as bass
import concourse.mybir as mybir
import concourse.tile as tile
from concourse.bass_utils import run_bass_kernel_spmd
from concourse.masks import make_identity

P = 128
NCORES = 8
BF = ml_dtypes.bfloat16

FULL_CFG = dict(N=20000, E=640000, FIN=128, HID=64, HEADS=4, NG=256, OUT=32)

F32 = mybir.dt.float32
BF16 = mybir.dt.bfloat16
I16 = mybir.dt.int16

# layer-1 row layout (bf16): 4 x [h(64) | 1] then asrc(4) adst(4) pad -> 384
CW = 65                  # head group width (64 + denom column)
D1R = 4 * CW             # 260
ASRC1, ADST1 = D1R, D1R + 4
ROW1 = 384
# layer-2 row layout (bf16): [h(64) | 1 | asrc(1) | adst(1) | pad] -> 128
ASRC2, ADST2 = CW, CW + 1
ROW2 = 128


# ---------------------------------------------------------------------------
# Host-side preparation: integer index manipulation + array reordering only.
# ---------------------------------------------------------------------------
def prepare(inputs, cfg):
    N, E, FIN, HID, HEADS, NG, OUT = (
        cfg["N"], cfg["E"], cfg["FIN"], cfg["HID"], cfg["HEADS"], cfg["NG"],
        cfg["OUT"],
    )
    GPC = NG // NCORES  # graphs per core

    x = np.asarray(inputs["x"], np.float32)
    ei = np.asarray(inputs["edge_index"], np.int64)
    ea = np.asarray(inputs["edge_attr"], np.float32)
    batch = np.asarray(inputs["batch"], np.int64)
    src, dst = ei[0], ei[1]

    # node ranges per core (graph-aligned; batch is sorted)
    bounds = np.searchsorted(batch, np.arange(NCORES + 1) * GPC)
    node_cnt = np.diff(bounds)
    NT = max(1, math.ceil(node_cnt.max() / P))
    NSLICE = NT * P
    NROWS = NCORES * NSLICE
    assert NROWS < 32768, f"int16 gather index overflow: {NROWS}"

    core_of_node = np.minimum(batch // GPC, NCORES - 1).astype(np.int64)
    rowid = np.empty(N, np.int64)
    for c in range(NCORES):
        ns, ne = bounds[c], bounds[c + 1]
        rowid[ns:ne] = c * NSLICE + np.arange(ne - ns)

    # edges sorted by dst; core blocks are contiguous
    order = np.argsort(dst, kind="stable")
    dsts = dst[order]
    srcs = src[order]
    ws = ea[order, 0]
    ecore = core_of_node[dsts]
    ebounds = np.searchsorted(ecore, np.arange(NCORES + 1))

    # chunks-per-tile: max over all (core, tile), rounded up to even
    cpt_max = 1
    tile_edge_counts = []
    for c in range(NCORES):
        es, ee = ebounds[c], ebounds[c + 1]
        dln = dsts[es:ee] - bounds[c]
        tid = dln // P
        cnts = np.bincount(tid, minlength=NT)
        tile_edge_counts.append(cnts)
        if len(cnts):
            cpt_max = max(cpt_max, math.ceil(cnts.max() / P))
    CPT = cpt_max + (cpt_max % 2)  # even
    CPT = max(CPT, 2)
    CH = CPT // 2
    NCHUNK = NT * CPT

    # per-layer-1/2 attention-edge scalars (tiny float prep, host-replicated)
    q1 = (np.asarray(inputs["We1"], np.float32).reshape(HEADS, HID)
          * np.asarray(inputs["att_edge1"], np.float32)).sum(axis=1)  # [H]
    q2 = float((np.asarray(inputs["We2"], np.float32).reshape(-1)
                * np.asarray(inputs["att_edge2"], np.float32).reshape(-1))
               .sum())

    jj = np.arange(P, dtype=np.int64)

    per_core = []
    for c in range(NCORES):
        ns, ne = bounds[c], bounds[c + 1]
        es, ee = ebounds[c], ebounds[c + 1]
        nloc = ne - ns

        xs = np.zeros((NSLICE, FIN), BF)
        xs[:nloc] = x[ns:ne].astype(BF)

        srcrow = np.zeros((NT, CPT * P), np.int64)
        dstl = np.full((NT, CPT * P), -1, np.int64)
        wv = np.zeros((NT, CPT * P), np.float32)

        dln = dsts[es:ee] - ns
        tid = dln // P
        cnts = tile_edge_counts[c]
        off = np.zeros(NT + 1, np.int64)
        off[1:NT + 1] = np.cumsum(cnts[:NT])
        for t in range(NT):
            k = int(cnts[t]) if t < len(cnts) else 0
            if k == 0:
                continue
            sel = slice(es + int(off[t]), es + int(off[t]) + k)
            srcrow[t, :k] = rowid[srcs[sel]]
            dstl[t, :k] = dln[int(off[t]):int(off[t]) + k] % P
            wv[t, :k] = ws[sel]

        # one-hot constants: eq[k, e, j] = (dstl[k*128+e] == j); fp8e4 (exact
        # 0/1) halves the constant DMA traffic vs bf16
        dstl_f = dstl.reshape(NCHUNK, P)
        eq = (dstl_f[:, :, None] == jj[None, None, :])
        oh_dev = np.ascontiguousarray(
            eq.transpose(1, 0, 2).reshape(P, NCHUNK * P)).astype(
                ml_dtypes.float8_e4m3)
        ohT_dev = np.ascontiguousarray(
            eq.transpose(2, 0, 1).reshape(P, NCHUNK * P)).astype(
                ml_dtypes.float8_e4m3)

        # per-edge attention-edge terms (w_e * q_h); pad slots -> 0
        wflat = wv.reshape(NCHUNK, P)
        ae1 = np.ascontiguousarray(
            (wflat[:, :, None] * q1[None, None, :]).transpose(1, 0, 2)
        ).astype(BF)                                   # [128, NCHUNK, H]
        ae2 = np.ascontiguousarray(
            (wflat * q2).transpose(1, 0)).astype(BF)   # [128, NCHUNK]

        def wrap_idx(arr):  # [NT, CPT*P] -> [128, NT*CPT*8] int16
            blocks = []
            for t in range(NT):
                for h in range(2):
                    ids = arr[t, h * CH * P:(h + 1) * CH * P]
                    a = ids.reshape(CH * 8, 16).T  # [16, CH*8]
                    blocks.append(np.tile(a, (8, 1)))
            return np.ascontiguousarray(
                np.concatenate(blocks, axis=1)).astype(np.int16)

        # per-(tile, half, sub-call) gather counts: sub-calls of 8 and 9
        # chunks; pad slots beyond the count are never gathered (their
        # one-hot columns are zero so the stale data cannot contribute)
        ecnt = np.zeros((NT, 2, 2), np.uint32)
        for t in range(NT):
            k = int(tile_edge_counts[c][t]) if t < len(tile_edge_counts[c]) \
                else 0
            for hh in range(2):
                h = min(max(k - hh * CH * P, 0), CH * P)
                ecnt[t, hh, 0] = min(max(h, 16), 8 * P)
                ecnt[t, hh, 1] = min(max(h - 8 * P, 16), (CH - 8) * P)

        # pooling one-hot + 1/count (host: index data only)
        bl = np.full((NSLICE,), -1, np.int64)
        bl[:nloc] = batch[ns:ne] - c * GPC
        poolg = np.ascontiguousarray(
            (bl.reshape(NT, P)[:, :, None] ==
             np.arange(GPC)[None, None, :]).transpose(1, 0, 2)).astype(BF)
        cnt = np.bincount(bl[:nloc], minlength=GPC).astype(np.float32)
        invc = (1.0 / np.maximum(cnt, 1.0)).reshape(GPC, 1).astype(np.float32)

        per_core.append(dict(
            xs=xs, idxs=wrap_idx(srcrow), oh=oh_dev, ohT=ohT_dev,
            ae1=ae1, ae2=ae2, poolg=poolg, invc=invc,
            ecnt=ecnt.reshape(1, -1).copy(),
        ))

    # weight-side constants (tiny, host-replicated)
    W1 = np.asarray(inputs["W1"], np.float32)            # [FIN, H*HID]
    W2 = np.asarray(inputs["W2"], np.float32)            # [H*HID, HID]
    rep = lambda vv: np.tile(np.asarray(vv, np.float32).reshape(1, -1),
                             (P, 1)).copy()
    consts = dict(
        W1b=W1.astype(BF),
        W2b=np.ascontiguousarray(
            W2.reshape(2, P, HID).transpose(1, 0, 2)).astype(BF),
        as1b=rep(inputs["att_src1"]), ad1b=rep(inputs["att_dst1"]),
        b1b=rep(inputs["b1"]),
        as2b=rep(inputs["att_src2"]), ad2b=rep(inputs["att_dst2"]),
        b2b=rep(inputs["b2"]),
        fcw=np.asarray(inputs["fcW"], np.float32),
        fcbb=rep(inputs["fcb"]),
    )

    in_maps = []
    for c in range(NCORES):
        m = dict(per_core[c])
        m.update(consts)
        in_maps.append(m)

    meta = dict(NT=NT, CPT=CPT, CH=CH, NSLICE=NSLICE, NROWS=NROWS,
                GPC=GPC, NCHUNK=NCHUNK, **cfg)
    return in_maps, meta


# ---------------------------------------------------------------------------
# Device program.
# ---------------------------------------------------------------------------
def build(meta, reps=1, num_devices=NCORES):
    NT, CPT, CH = meta["NT"], meta["CPT"], meta["CH"]
    NSLICE, NROWS, GPC = meta["NSLICE"], meta["NROWS"], meta["GPC"]
    FIN, HID, HEADS, OUT = meta["FIN"], meta["HID"], meta["HEADS"], meta["OUT"]
    NCHUNK = meta["NCHUNK"]
    D1 = HEADS * HID          # 256
    NI = CH * P               # idxs per gather call
    NIc = NI // 16            # idx columns per call
    A = mybir.AluOpType
    ACT = mybir.ActivationFunctionType
    X = mybir.AxisListType.X
    rg = [list(range(NCORES))]

    nc = bacc.Bacc("TRN2", target_bir_lowering=False, debug=False,
                   num_devices=num_devices,
                   dynamic_dma_scratch_size=122880,
                   num_swdge_queues=4)

    def din(name, shape, dtype=F32):
        return nc.dram_tensor(name, list(shape), dtype,
                              kind="ExternalInput").ap()

    F8 = mybir.dt.float8e4
    xs = din("xs", (NSLICE, FIN), BF16)
    idxs_d = din("idxs", (P, NCHUNK * 8), I16)
    oh_d = din("oh", (P, NCHUNK * P), F8)
    ohT_d = din("ohT", (P, NCHUNK * P), F8)
    ae1_d = din("ae1", (P, NCHUNK * HEADS), BF16)
    ae2_d = din("ae2", (P, NCHUNK), BF16)
    poolg_d = din("poolg", (P, NT * GPC), BF16)
    invc_d = din("invc", (GPC, 1))
    W1_d = din("W1b", (FIN, D1), BF16)
    W2_d = din("W2b", (P, 2 * HID), BF16)
    as1_d = din("as1b", (P, D1))
    ad1_d = din("ad1b", (P, D1))
    b1_d = din("b1b", (P, D1))
    as2_d = din("as2b", (P, HID))
    ad2_d = din("ad2b", (P, HID))
    b2_d = din("b2b", (P, HID))
    fcw_d = din("fcw", (HID, OUT))
    fcb_d = din("fcbb", (P, OUT))

    out_d = nc.dram_tensor("out", [GPC, OUT], F32, kind="ExternalOutput").ap()

    gq = [0]  # rotating SWDGE queue
    SUBS = [(0, 8), (8, CH - 8)]  # sub-call (chunk offset, chunk count)

    with tile.TileContext(nc) as tc, ExitStack() as st:
        constp = st.enter_context(tc.tile_pool(name="constp", bufs=1))
        drp = st.enter_context(tc.tile_pool(name="drp", bufs=1, space="DRAM"))

        identf = constp.tile([P, P], F32)
        make_identity(nc, identf[:])
        identb = constp.tile([P, P], BF16)
        make_identity(nc, identb[:])
        ixs_all = constp.tile([P, NCHUNK * 8], I16)
        nc.sync.dma_start(ixs_all[:], idxs_d[:])
        ae1_sb = constp.tile([P, NCHUNK, HEADS], BF16)
        nc.sync.dma_start(ae1_sb[:],
                          ae1_d[:].rearrange("p (k h) -> p k h", h=HEADS))
        ae2_sb = constp.tile([P, NCHUNK], BF16)
        nc.sync.dma_start(ae2_sb[:], ae2_d[:])
        poolg_sb = constp.tile([P, NT, GPC], BF16)
        nc.sync.dma_start(poolg_sb[:],
                          poolg_d[:].rearrange("p (t g) -> p t g", g=GPC))
        invc_sb = constp.tile([GPC, 1], F32)
        nc.sync.dma_start(invc_sb[:], invc_d[:])
        b1_sb = constp.tile([P, D1], F32)
        nc.sync.dma_start(b1_sb[:], b1_d[:])
        def gather_sub(G, full, t, hh, row):
            """Issue the half-tile gather as 2 ring-sized sub-calls so a
            whole call fits a SWDGE ring and the first chunks' matmuls can
            start while the rest still streams."""
            gbase = (t * 2 + hh) * NIc
            for si, (co, cn) in enumerate(SUBS):
                nc.gpsimd.dma_gather(
                    G[:, co:co + cn, :], full[:],
                    ixs_all[:, gbase + co * 8:gbase + (co + cn) * 8],
                    cn * P, cn * P, row, single_packet=False,
                    queue_num=gq[0] % 4)
                gq[0] += 1

        for _rep in range(reps):
            t1loc = drp.tile([NSLICE, ROW1], BF16, name=f"t1loc{_rep}")
            t1full = drp.tile([NROWS, ROW1], BF16, addr_space="Shared",
                              name=f"t1full{_rep}")
            t2loc = drp.tile([NSLICE, ROW2], BF16, name=f"t2loc{_rep}")
            t2full = drp.tile([NROWS, ROW2], BF16, addr_space="Shared",
                              name=f"t2full{_rep}")

            adst1_all = constp.tile([P, NT, HEADS], BF16,
                                    name=f"adst1_{_rep}")
            adst2_all = constp.tile([P, NT, 1], BF16, name=f"adst2_{_rep}")
            out1 = constp.tile([P, NT, D1], BF16, name=f"out1_{_rep}")

            # ------------- Phase 0: h1 = x @ W1, logits, table1 -------------
            with tc.tile_pool(name="ph0", bufs=1) as sp, \
                 tc.tile_pool(name="ph0b", bufs=2) as sp2, \
                 tc.tile_pool(name="ph0p", bufs=2, space="PSUM") as pp:
                w1_sb = sp.tile([P, D1], BF16)
                nc.sync.dma_start(w1_sb[:], W1_d[:])
                as1_sb = sp.tile([P, D1], F32)
                nc.sync.dma_start(as1_sb[:], as1_d[:])
                ad1_sb = sp.tile([P, D1], F32)
                nc.sync.dma_start(ad1_sb[:], ad1_d[:])
                xT_all = sp.tile([P, NSLICE], BF16)
                nc.sync.dma_start_transpose(xT_all[:], xs[:])
                for t in range(NT):
                    h_ps = pp.tile([P, D1], F32, space="PSUM")
                    nc.tensor.matmul(h_ps[:],
                                     lhsT=xT_all[:, t * P:(t + 1) * P],
                                     rhs=w1_sb[:], start=True, stop=True)
                    tmp = sp2.tile([P, D1], F32)
                    red = sp2.tile([P, HEADS], F32)
                    nc.vector.tensor_tensor(out=tmp[:], in0=h_ps[:],
                                            in1=as1_sb[:], op=A.mult)
                    nc.vector.tensor_reduce(
                        out=red[:],
                        in_=tmp[:].rearrange("p (h f) -> p h f", h=HEADS),
                        axis=X, op=A.add)
                    t1t = sp2.tile([P, ROW1], BF16)
                    nc.vector.tensor_copy(out=t1t[:, ASRC1:ASRC1 + HEADS],
                                          in_=red[:])
                    nc.vector.tensor_tensor(out=tmp[:], in0=h_ps[:],
                                            in1=ad1_sb[:], op=A.mult)
                    nc.vector.tensor_reduce(
                        out=red[:],
                        in_=tmp[:].rearrange("p (h f) -> p h f", h=HEADS),
                        axis=X, op=A.add)
                    nc.vector.tensor_copy(out=adst1_all[:, t, :], in_=red[:])
                    nc.vector.tensor_copy(out=t1t[:, ADST1:ADST1 + HEADS],
                                          in_=red[:])
                    hv = t1t[:, 0:D1R].rearrange("p (h f) -> p h f", f=CW)
                    nc.vector.tensor_copy(
                        out=hv[:, :, 0:HID],
                        in_=h_ps[:].rearrange("p (h f) -> p h f", f=HID))
                    nc.vector.memset(hv[:, :, HID:CW], 1.0)
                    nc.vector.memset(t1t[:, ADST1 + HEADS:ROW1], 0.0)
                    nc.sync.dma_start(t1loc[t * P:(t + 1) * P, :], t1t[:])
                nc.gpsimd.collective_compute(
                    "AllGather", A.bypass, replica_groups=rg,
                    ins=[t1loc[:]], outs=[t1full[:]])

            # ------------- Phase 1+2: layer-1 edge phase fused with the
            # per-tile layer-2 GEMM/table build (overlaps AG2 prep) ----------
            with tc.tile_pool(name="p1g", bufs=4) as pg, \
                 tc.tile_pool(name="p1o", bufs=2) as po, \
                 tc.tile_pool(name="p1w", bufs=2) as pw, \
                 tc.tile_pool(name="p1c", bufs=1) as p1c, \
                 tc.tile_pool(name="p1b", bufs=2) as sp2, \
                 tc.tile_pool(name="p1ps", bufs=2, space="PSUM") as pps, \
                 tc.tile_pool(name="p1pa", bufs=2, space="PSUM") as ppa, \
                 tc.tile_pool(name="p1p2", bufs=1, space="PSUM") as pp2:
                w2_sb = p1c.tile([P, 2, HID], BF16)
                nc.sync.dma_start(w2_sb[:],
                                  W2_d[:].rearrange("p (k n) -> p k n", k=2))
                as2_sb = p1c.tile([P, HID], F32)
                nc.sync.dma_start(as2_sb[:], as2_d[:])
                ad2_sb = p1c.tile([P, HID], F32)
                nc.sync.dma_start(ad2_sb[:], ad2_d[:])
                for t in range(NT):
                    acc = pps.tile([P, D1R], F32, space="PSUM")
                    for hh in range(2):
                        kb = t * CPT + hh * CH       # chunk base
                        G = pg.tile([P, CH, ROW1], BF16)
                        gather_sub(G, t1full, t, hh, ROW1)
                        oh_sb = po.tile([P, CH, P], F8)
                        nc.sync.dma_start(
                            oh_sb[:], oh_d[:, kb * P:(kb + CH) * P]
                            .rearrange("p (c j) -> p c j", j=P))
                        ohT_sb = po.tile([P, CH, P], F8)
                        nc.sync.dma_start(
                            ohT_sb[:], ohT_d[:, kb * P:(kb + CH) * P]
                            .rearrange("p (c j) -> p c j", j=P))
                        adp_ps = ppa.tile([P, CH, HEADS], F32, space="PSUM")
                        for c in range(CH):
                            nc.tensor.matmul(adp_ps[:, c, :],
                                             lhsT=ohT_sb[:, c, :],
                                             rhs=adst1_all[:, t, :],
                                             start=True, stop=True)
                        alpha = pw.tile([P, CH, HEADS], F32)
                        nc.vector.tensor_tensor(
                            out=alpha[:], in0=G[:, :, ASRC1:ASRC1 + HEADS],
                            in1=adp_ps[:], op=A.add)
                        nc.vector.tensor_tensor(
                            out=alpha[:], in0=alpha[:],
                            in1=ae1_sb[:, kb:kb + CH, :], op=A.add)
                        e2 = pw.tile([P, CH, HEADS], F32)
                        nc.scalar.activation(out=e2[:], in_=alpha[:],
                                             func=ACT.Exp, scale=0.2)
                        nc.scalar.activation(out=alpha[:], in_=alpha[:],
                                             func=ACT.Exp)
                        p_bf = pw.tile([P, CH, HEADS], BF16)
                        nc.vector.tensor_tensor(out=p_bf[:], in0=alpha[:],
                                                in1=e2[:], op=A.max)
                        gv = G[:, :, 0:D1R].rearrange(
                            "p c (h f) -> p c h f", f=CW)
                        nc.vector.tensor_tensor(
                            out=gv, in0=gv,
                            in1=p_bf[:].unsqueeze(3)
                                .to_broadcast([P, CH, HEADS, CW]),
                            op=A.mult)
                        for c in range(CH):
                            nc.tensor.matmul(
                                acc[:], lhsT=oh_sb[:, c, :],
                                rhs=G[:, c, 0:D1R],
                                start=(hh == 0 and c == 0),
                                stop=(hh == 1 and c == CH - 1))
                    # epilogue: out1 = relu(acc_h / denom_h + b1), fp32 ops
                    # then one contiguous cast (strided bf16 DVE writes
                    # measured pathologically slow on HW)
                    accv = acc[:].rearrange("p (h f) -> p h f", f=CW)
                    dn = pw.tile([P, HEADS], F32)
                    nc.vector.tensor_scalar(out=dn[:], in0=accv[:, :, HID],
                                            scalar1=1e-16, scalar2=None,
                                            op0=A.add)
                    rc = pw.tile([P, HEADS], F32)
                    nc.vector.reciprocal(rc[:], dn[:])
                    o1 = pw.tile([P, HEADS, HID], F32)
                    nc.vector.tensor_tensor(
                        out=o1[:], in0=accv[:, :, 0:HID],
                        in1=rc[:].unsqueeze(2).to_broadcast([P, HEADS, HID]),
                        op=A.mult)
                    nc.vector.tensor_tensor(
                        out=o1[:], in0=o1[:],
                        in1=b1_sb[:].rearrange("p (h f) -> p h f", h=HEADS),
                        op=A.add)
                    nc.scalar.activation(
                        out=out1[:, t, :],
                        in_=o1[:].rearrange("p h f -> p (h f)"),
                        func=ACT.Relu)

                    # layer-2 GEMM + table row for this tile (fused phase 2)
                    h2_ps = pp2.tile([P, HID], F32, space="PSUM")
                    for k in range(2):
                        hT_ps = pp2.tile([P, P], BF16, space="PSUM")
                        nc.tensor.transpose(
                            hT_ps[:], out1[:, t, k * P:(k + 1) * P],
                            identb[:])
                        hT = sp2.tile([P, P], BF16)
                        nc.vector.tensor_copy(out=hT[:], in_=hT_ps[:])
                        nc.tensor.matmul(h2_ps[:], lhsT=hT[:],
                                         rhs=w2_sb[:, k, :],
                                         start=(k == 0), stop=(k == 1))
                    t2t = sp2.tile([P, ROW2], BF16)
                    tmp = sp2.tile([P, HID], F32)
                    red1 = sp2.tile([P, 1], F32)
                    nc.vector.tensor_tensor(out=tmp[:], in0=h2_ps[:],
                                            in1=as2_sb[:], op=A.mult)
                    nc.vector.tensor_reduce(out=red1[:],
                                            in_=tmp[:], axis=X, op=A.add)
                    nc.vector.tensor_copy(out=t2t[:, ASRC2:ASRC2 + 1],
                                          in_=red1[:])
                    nc.vector.tensor_tensor(out=tmp[:], in0=h2_ps[:],
                                            in1=ad2_sb[:], op=A.mult)
                    red2 = sp2.tile([P, 1], F32)
                    nc.vector.tensor_reduce(out=red2[:],
                                            in_=tmp[:], axis=X, op=A.add)
                    nc.vector.tensor_copy(out=adst2_all[:, t, :], in_=red2[:])
                    nc.vector.tensor_copy(out=t2t[:, ADST2:ADST2 + 1],
                                          in_=red2[:])
                    nc.vector.tensor_copy(out=t2t[:, 0:HID], in_=h2_ps[:])
                    nc.vector.memset(t2t[:, HID:HID + 1], 1.0)
                    nc.vector.memset(t2t[:, ADST2 + 1:ROW2], 0.0)
                    nc.sync.dma_start(t2loc[t * P:(t + 1) * P, :], t2t[:])
                nc.gpsimd.collective_compute(
                    "AllGather", A.bypass, replica_groups=rg,
                    ins=[t2loc[:]], outs=[t2full[:]])

            # ------------- Phase 3: layer-2 edge phase + pooling ------------
            with tc.tile_pool(name="p3g", bufs=4) as pg, \
                 tc.tile_pool(name="p3o", bufs=3) as po, \
                 tc.tile_pool(name="p3w", bufs=2) as pw, \
                 tc.tile_pool(name="p3c", bufs=1) as pc, \
                 tc.tile_pool(name="p3ps", bufs=2, space="PSUM") as pps, \
                 tc.tile_pool(name="p3pa", bufs=2, space="PSUM") as ppa, \
                 tc.tile_pool(name="p3f", bufs=1, space="PSUM") as ppf, \
                 tc.tile_pool(name="p3pl", bufs=1, space="PSUM") as ppl:
                b2_sb = pc.tile([P, HID], F32)
                nc.sync.dma_start(b2_sb[:], b2_d[:])
                pool_ps = ppl.tile([GPC, HID], F32, space="PSUM")
                for t in range(NT):
                    acc = pps.tile([P, CW], F32, space="PSUM")
                    for hh in range(2):
                        kb = t * CPT + hh * CH
                        G = pg.tile([P, CH, ROW2], BF16)
                        gather_sub(G, t2full, t, hh, ROW2)
                        oh_sb = po.tile([P, CH, P], F8)
                        nc.sync.dma_start(
                            oh_sb[:], oh_d[:, kb * P:(kb + CH) * P]
                            .rearrange("p (c j) -> p c j", j=P))
                        ohT_sb = po.tile([P, CH, P], F8)
                        nc.sync.dma_start(
                            ohT_sb[:], ohT_d[:, kb * P:(kb + CH) * P]
                            .rearrange("p (c j) -> p c j", j=P))
                        adp_ps = ppa.tile([P, CH, 1], F32, space="PSUM")
                        for c in range(CH):
                            nc.tensor.matmul(adp_ps[:, c, :],
                                             lhsT=ohT_sb[:, c, :],
                                             rhs=adst2_all[:, t, :],
                                             start=True, stop=True)
                        alpha = pw.tile([P, CH, 1], F32)
                        nc.vector.tensor_tensor(
                            out=alpha[:], in0=G[:, :, ASRC2:ASRC2 + 1],
                            in1=adp_ps[:], op=A.add)
                        nc.vector.tensor_tensor(
                            out=alpha[:], in0=alpha[:],
                            in1=ae2_sb[:, kb:kb + CH].unsqueeze(2),
                            op=A.add)
                        e2 = pw.tile([P, CH, 1], F32)
                        nc.scalar.activation(out=e2[:], in_=alpha[:],
                                             func=ACT.Exp, scale=0.2)
                        nc.scalar.activation(out=alpha[:], in_=alpha[:],
                                             func=ACT.Exp)
                        p_bf = pw.tile([P, CH, 1], BF16)
                        nc.vector.tensor_tensor(out=p_bf[:], in0=alpha[:],
                                                in1=e2[:], op=A.max)
                        gv = G[:, :, 0:CW]
                        nc.vector.tensor_tensor(
                            out=gv, in0=gv,
                            in1=p_bf[:].to_broadcast([P, CH, CW]),
                            op=A.mult)
                        for c in range(CH):
                            nc.tensor.matmul(
                                acc[:], lhsT=oh_sb[:, c, :],
                                rhs=G[:, c, 0:CW],
                                start=(hh == 0 and c == 0),
                                stop=(hh == 1 and c == CH - 1))
                    # epilogue: o2 = relu(acc/denom + b2) -> bf16, pool matmul
                    dn = pw.tile([P, 1], F32)
                    nc.vector.tensor_scalar(out=dn[:], in0=acc[:, HID:CW],
                                            scalar1=1e-16, scalar2=None,
                                            op0=A.add)
                    rc = pw.tile([P, 1], F32)
                    nc.vector.reciprocal(rc[:], dn[:])
                    o2 = pw.tile([P, HID], F32)
                    nc.vector.tensor_scalar(out=o2[:], in0=acc[:, 0:HID],
                                            scalar1=rc[:, 0:1], scalar2=None,
                                            op0=A.mult)
                    nc.vector.tensor_tensor(out=o2[:], in0=o2[:],
                                            in1=b2_sb[:], op=A.add)
                    o2b = pw.tile([P, HID], BF16)
                    nc.scalar.activation(out=o2b[:], in_=o2[:],
                                         func=ACT.Relu)
                    nc.tensor.matmul(pool_ps[:], lhsT=poolg_sb[:, t, :],
                                     rhs=o2b[:], start=(t == 0),
                                     stop=(t == NT - 1),
                                     skip_group_check=True)

                # ------------- Phase 4: pooled mean + FC --------------------
                fcw_sb = pc.tile([HID, OUT], F32)
                nc.sync.dma_start(fcw_sb[:], fcw_d[:])
                fcb_sb = pc.tile([P, OUT], F32)
                nc.sync.dma_start(fcb_sb[:], fcb_d[:])
                pooled = pc.tile([GPC, HID], F32)
                nc.vector.tensor_scalar(out=pooled[:], in0=pool_ps[:],
                                        scalar1=invc_sb[:, 0:1], scalar2=None,
                                        op0=A.mult)
                pT_ps = ppf.tile([HID, GPC], F32, space="PSUM")
                nc.tensor.transpose(pT_ps[:], pooled[:], identf[:GPC, :GPC])
                pT = pc.tile([HID, GPC], F32)
                nc.vector.tensor_copy(out=pT[:], in_=pT_ps[:])
                fc_ps = ppf.tile([GPC, OUT], F32, space="PSUM")
                nc.tensor.matmul(fc_ps[:], lhsT=pT[:], rhs=fcw_sb[:],
                                 start=True, stop=True)
                res = pc.tile([GPC, OUT], F32)
                nc.vector.tensor_tensor(out=res[:], in0=fc_ps[:],
                                        in1=fcb_sb[:GPC, :], op=A.add)
                nc.sync.dma_start(out_d[:], res[:])

    nc.compile()
    return nc


# ---------------------------------------------------------------------------
# Entry point.
# ---------------------------------------------------------------------------
def run(inputs, cfg, **run_kwargs):
    in_maps, meta = prepare(inputs, cfg)
    nc = build(meta)
    res = run_bass_kernel_spmd(nc, in_maps, core_ids=list(range(NCORES)),
                               **run_kwargs)
    out = np.concatenate([res.results[c]["out"] for c in range(NCORES)],
                         axis=0)
    return np.asarray(out, np.float32), res


def kernel(**inputs) -> np.ndarray:
    out, _ = run(inputs, FULL_CFG)
    return out
